# revision 36
# baseline (speedup 1.0000x reference)
"""Trainium2 Bass kernel for nn_DualBranchDecoder.

Dual-branch band-split decoder: per-band GroupNorm -> fc1(C=128->H=512)+tanh
-> per-band fc2(H->w_k) -> sigmoid mag mask / tanh phase offset -> complex out.

Sharding: data-parallel over batch B=8 across 8 NeuronCores (one sample per
core).

Design notes:
- feat host-cast to fp16. GroupNorm folded: feat is pre-scaled by inv on DVE
  (fp16 4x mode), the mean term goes into the fc1 activation bias
  (b1p - inv*mu*S1g), so fc1 tanh needs only a bias AP (a scale AP costs
  ~83ns/activation extra on the scalar engine).
- sigmoid rewritten as 0.5 + 0.5*tanh(x/2): the whole kernel runs on the
  tanh table until the finale's Sin; the affine folds into the finale.
- per-quad stats (bn_stats -> cross-partition reduce -> quake-rsqrt Newton)
  batched across both branches; quad 0 runs per-branch chains to shorten
  the head.
- finale: one batched block over [128, 2T] fp16 at the end; fp16 outputs
  (host upcasts). Freq row 256 assembled on host from the aux mask rows.
"""
import sys
sys.path.insert(0, '/opt/trn_rl_repo')

import numpy as np
import ml_dtypes

import concourse.bacc as bacc
import concourse.tile as tile
import concourse.mybir as mybir
from concourse.bass_utils import run_bass_kernel_spmd

F32 = mybir.dt.float32
FP16 = mybir.dt.float16
I32 = mybir.dt.int32
AF = mybir.ActivationFunctionType
ALU = mybir.AluOpType

# problem constants (hardcoded per contract)
B, C, T = 8, 128, 512
BANDS = [2] + [3] * 10 + [8] * 12 + [16] * 7 + [17]
K = len(BANDS)                      # 31
F = sum(BANDS)                      # 257
H = 4 * C                           # 512
NHC = H // 128                      # 4 h-chunks
EPS = 1e-5

OFFS = np.concatenate([[0], np.cumsum(BANDS)]).astype(int)   # band start freqs
WPADS = [w + (w & 1) for w in BANDS]                         # even-M pad
WOFFS = np.concatenate([[0], np.cumsum(WPADS)]).astype(int)
WPTOT = int(WOFFS[-1])

QUADS = [(4 * i, 4) for i in range(7)] + [(28, 3)]
NQ = len(QUADS)
MAGIC16 = 1536.0                     # 1.5 * 2**10: fp16 round-to-int magic
INV2PI = float(1.0 / (2 * np.pi))
N2PI = float(-2 * np.pi)
PI = float(np.pi)
N_WARM = 6

_cache = {}


def _prep_branch(gamma, beta, W1, b1, W2, b2, is_mag):
    """Host-side constant prep for one branch."""
    # W1gT[c, k*H + h] = W1[k,h,c] * gamma[k,c]
    W1g = W1 * gamma[:, None, :]                      # [K, H, C]
    W1gT = np.ascontiguousarray(W1g.transpose(2, 0, 1).reshape(C, K * H))
    W1gT16 = W1gT.astype(np.float16)
    # b1p[k,h] = b1[k,h] + sum_c W1[k,h,c]*beta[k,c];  layout [128, K*NHC]
    b1p = b1 + np.einsum('khc,kc->kh', W1, beta)      # [K, H]
    b1pT = np.zeros((128, K * NHC), np.float32)
    # ns1gt[p, k*NHC+hc] = -sum_c W1g16[c, k, hc*128+p]
    ns1gt = np.zeros((128, K * NHC), np.float32)
    W1gT16f = W1gT16.astype(np.float32)
    for k in range(K):
        for hc in range(NHC):
            b1pT[:, k * NHC + hc] = b1p[k, hc * 128:(hc + 1) * 128]
            ns1gt[:, k * NHC + hc] = -W1gT16f[:, k * H + hc * 128:
                                              k * H + (hc + 1) * 128].sum(axis=0)
    # mag branch computes y = tanh(0.5*(fc2 + b2)): fold 0.5 into W2 and b2
    wscale = 0.5 if is_mag else 1.0
    W2Tp = np.zeros((128, NHC * WPTOT), np.float32)
    for k in range(K):
        w, off, woff = BANDS[k], OFFS[k], WOFFS[k]
        for hc in range(NHC):
            W2Tp[:, hc * WPTOT + woff: hc * WPTOT + woff + w] = \
                wscale * W2[off:off + w, hc * 128:(hc + 1) * 128].T
    W2Tp = W2Tp.astype(np.float16)
    b2g = np.zeros((128, len(QUADS)), np.float32)
    for q, (k0, nb) in enumerate(QUADS):
        for r in range(nb):
            k = k0 + r
            b2g[32 * r:32 * r + BANDS[k], q] = \
                wscale * b2[OFFS[k]:OFFS[k] + BANDS[k]]
    return W1gT16, b1pT, ns1gt, W2Tp, b2g


def _build():
    nc = bacc.Bacc("TRN2", target_bir_lowering=False)

    # per-core inputs
    ins = {}
    for br in ("m", "p"):
        ins[f"feat_{br}"] = nc.dram_tensor(f"feat_{br}", [C, K * T], FP16,
                                           kind="ExternalInput")
        ins[f"w1gt_{br}"] = nc.dram_tensor(f"w1gt_{br}", [C, K * H], FP16,
                                           kind="ExternalInput")
        ins[f"b1pt_{br}"] = nc.dram_tensor(f"b1pt_{br}", [128, K * NHC], F32,
                                           kind="ExternalInput")
        ins[f"ns1gt_{br}"] = nc.dram_tensor(f"ns1gt_{br}", [128, K * NHC], F32,
                                            kind="ExternalInput")
        ins[f"w2tp_{br}"] = nc.dram_tensor(f"w2tp_{br}", [128, NHC * WPTOT],
                                           FP16, kind="ExternalInput")
        ins[f"b2c_{br}"] = nc.dram_tensor(f"b2c_{br}", [128, len(QUADS)], F32,
                                          kind="ExternalInput")
        ins[f"noisy_{br}"] = nc.dram_tensor(f"noisy_{br}", [2 * 128, T], FP16,
                                            kind="ExternalInput")
    halfpi_d = nc.dram_tensor("halfpi", [128, 1], F32, kind="ExternalInput")
    out_d = nc.dram_tensor("out", [2 * 128, 2 * T], FP16,
                           kind="ExternalOutput")
    aux_d = nc.dram_tensor("aux", [2, T], FP16, kind="ExternalOutput")

    with tile.TileContext(nc) as tc:
        with (
            tc.tile_pool(name="featk", bufs=3) as featk_pool,
            tc.tile_pool(name="fsc", bufs=3) as fsc_pool,
            tc.tile_pool(name="w1t", bufs=3) as w1t_pool,
            tc.tile_pool(name="h1sb", bufs=3) as h1sb_pool,
            tc.tile_pool(name="band", bufs=4) as band_pool,
            tc.tile_pool(name="const", bufs=1) as const_pool,
            tc.tile_pool(name="statsb", bufs=2) as stats_pool,
            tc.tile_pool(name="fin", bufs=1) as fin_pool,
            tc.tile_pool(name="mainps", bufs=1, space="PSUM") as main_ps,
        ):
            # ---- critical-path first: quad-0 fetches before anything ----
            k0_0, nb_0 = QUADS[0]
            eng_f = {"m": nc.sync, "p": nc.gpsimd}
            fq0 = {}
            wq0 = {}
            for br in ("m", "p"):
                fq0[br] = featk_pool.tile([128, nb_0 * T], FP16, tag="featq",
                                          name=f"featq_{br}_0")
                # per-band DMAs so band-0 stats can start early
                for r in range(nb_0):
                    eng_f[br].dma_start(
                        fq0[br][:, r * T:(r + 1) * T],
                        ins[f"feat_{br}"][:, (k0_0 + r) * T:(k0_0 + r + 1) * T])
                wq0[br] = w1t_pool.tile([128, nb_0 * H], FP16, tag="w1q",
                                        name=f"w1q_{br}_0")
                nc.scalar.dma_start(
                    wq0[br][:], ins[f"w1gt_{br}"][:, k0_0 * H:(k0_0 + nb_0) * H])

            # ---- all-ones stationary for the reduce-broadcast matmul ----
            # fp16 (with fp16 sums) keeps the per-quad ps_r matmul off the
            # slow f32 weight-load path (~600ns -> ~150ns)
            ones128 = const_pool.tile([128, 128], FP16, tag="ones128",
                                      name="ones128")
            nc.vector.memset(ones128[:], 1.0)
            # ---- PE warm-up from a memset tile (no DMA dependency) ----
            warm_sb = const_pool.tile([128, T], FP16, tag="warm_sb",
                                      name="warm_sb")
            nc.vector.memset(warm_sb[:], 0.001)
            for wi in range(N_WARM):
                wps = main_ps.tile([128, T], F32, tag="h1ps", bufs=5,
                                   name=f"warm_{wi}")
                nc.tensor.matmul(wps[:], warm_sb[:, 0:128], warm_sb[:],
                                 start=True, stop=True)

            cb = {}
            noisy = {}
            for br in ("m", "p"):
                b1pt = const_pool.tile([128, K * NHC], F32, tag=f"b1pt_{br}",
                                       name=f"b1pt_{br}")
                nc.scalar.dma_start(b1pt[:], ins[f"b1pt_{br}"][:])
                ns1gt = const_pool.tile([128, K * NHC], F32, tag=f"ns1gt_{br}",
                                        name=f"ns1gt_{br}")
                nc.scalar.dma_start(ns1gt[:], ins[f"ns1gt_{br}"][:])
                w2tp = const_pool.tile([128, NHC * WPTOT], FP16,
                                       tag=f"w2tp_{br}", name=f"w2tp_{br}")
                nc.gpsimd.dma_start(w2tp[:], ins[f"w2tp_{br}"][:])
                b2c = const_pool.tile([128, len(QUADS)], F32, tag=f"b2c_{br}",
                                      name=f"b2c_{br}")
                nc.sync.dma_start(b2c[:], ins[f"b2c_{br}"][:])
                cb[br] = (b1pt, ns1gt, w2tp, b2c)
                nz = const_pool.tile([128, 2 * T], FP16, tag=f"noisy_{br}",
                                     name=f"noisy_{br}")
                noisy[br] = nz
            halfpi = const_pool.tile([128, 1], F32)

            # ---- masks (tanh outputs y), fp16 ----
            masks = {}
            for br in ("m", "p"):
                masks[br] = const_pool.tile([128, 2 * T], FP16,
                                            tag=f"mask_{br}", name=f"mask_{br}")
                masks[br + "2"] = const_pool.tile([1, T], FP16,
                                                  tag=f"mask2_{br}",
                                                  name=f"mask2_{br}")

            def stats_chain(q, k0, nb, fq, branches):
                """bn stats -> reduce-broadcast -> newton rsqrt -> prescaled
                feat + per-(band,hc) activation bias for `branches`.

                Returns ({br: fsc}, {br: biasq}).
                """
                nbr = len(branches)
                sfx = f"{q}_{branches[0]}"
                sums = stats_pool.tile([128, 2 * nb * nbr], FP16, tag="sums",
                                       bufs=3, name=f"sums_{sfx}")
                for bi, br in enumerate(branches):
                    o = 2 * nb * bi
                    st_q = stats_pool.tile([128, nb * 6], F32, tag="st_q",
                                           name=f"st_{br}_{q}")
                    ag_q = stats_pool.tile([128, nb * 2], F32, tag="ag_q",
                                           name=f"ag_{br}_{q}")
                    for r in range(nb):
                        nc.vector.bn_stats(st_q[:, r * 6:(r + 1) * 6],
                                           fq[br][:, r * T:(r + 1) * T])
                        nc.vector.bn_aggr(ag_q[:, r * 2:(r + 1) * 2],
                                          st_q[:, r * 6:(r + 1) * 6])
                    ag3 = ag_q[:].rearrange("c (k two) -> c k two", two=2)
                    mean_ap = ag3[:, :, 0]
                    var_ap = ag3[:, :, 1]
                    nc.vector.tensor_copy(sums[:, o:o + nb], mean_ap)
                    tmp = stats_pool.tile([128, nb], F32, tag="tmp",
                                          name=f"tmp_{br}_{q}")
                    nc.vector.tensor_mul(tmp[:], mean_ap, mean_ap)
                    nc.vector.tensor_add(sums[:, o + nb:o + 2 * nb], tmp[:],
                                         var_ap)
                # reduce + broadcast in one matmul: out[m, j] = sum_p sums[p,j]
                ps_r = main_ps.tile([128, 2 * nb * nbr], F32, tag="ps_s",
                                    bufs=1, name=f"ps_r_{sfx}")
                nc.tensor.matmul(ps_r[:], ones128[:], sums[:],
                                 start=True, stop=True)
                g = stats_pool.tile([128, 2 * nb * nbr], F32, tag="g",
                                    name=f"g_{sfx}")
                nc.vector.tensor_scalar_mul(g[:], ps_r[:], 1.0 / C)
                g4 = g[:].rearrange("o (b two n) -> o b two n", b=nbr, two=2)
                gmean = g4[:, :, 0, :]
                gsq = g4[:, :, 1, :]
                gm2 = stats_pool.tile([128, nb * nbr], F32, tag="gm2",
                                      name=f"gm2_{sfx}")
                nc.vector.tensor_mul(gm2[:], gmean, gmean)
                vv = stats_pool.tile([128, nb * nbr], F32, tag="vv",
                                     name=f"vv_{sfx}")
                nc.vector.tensor_sub(vv[:], gsq, gm2[:])
                nc.vector.tensor_scalar_add(vv[:], vv[:], EPS)
                yy = stats_pool.tile([128, nb * nbr], F32, tag="yy",
                                     name=f"yy_{sfx}")
                nc.vector.tensor_scalar(yy[:].bitcast(I32), vv[:].bitcast(I32),
                                        1, -1, op0=ALU.arith_shift_right,
                                        op1=ALU.bitwise_xor)
                nc.vector.tensor_scalar_add(yy[:].bitcast(I32),
                                            yy[:].bitcast(I32), 0x5f3759e0)
                bbq = stats_pool.tile([128, 2 * nb * nbr], F32, tag="bbq",
                                      bufs=3, name=f"bbq_{sfx}")
                iv4 = bbq[:].rearrange("o (b two n) -> o b two n",
                                       b=nbr, two=2)
                inv_ap = iv4[:, :, 0, :]
                invmu_ap = iv4[:, :, 1, :]
                tnr = stats_pool.tile([128, nb * nbr], F32, tag="tnr",
                                      name=f"tnr_{sfx}")
                for it in range(3):
                    nc.vector.tensor_mul(tnr[:], yy[:], yy[:])
                    nc.vector.tensor_mul(tnr[:], tnr[:], vv[:])
                    nc.vector.tensor_scalar(tnr[:], tnr[:], -0.5, 1.5,
                                            op0=ALU.mult, op1=ALU.add)
                    dst = yy[:] if it < 2 else inv_ap
                    nc.vector.tensor_mul(dst, yy[:], tnr[:])
                nc.vector.tensor_mul(invmu_ap, inv_ap, gmean)

                fscs = {}
                biasqs = {}
                for bi, br in enumerate(branches):
                    o = 2 * nb * bi
                    b1pt, ns1gt, _, _ = cb[br]
                    fsc = fsc_pool.tile([128, nb * T], FP16, tag="fsc",
                                        name=f"fsc_{br}_{q}")
                    for r in range(nb):
                        nc.vector.tensor_scalar_mul(
                            fsc[:, r * T:(r + 1) * T],
                            fq[br][:, r * T:(r + 1) * T],
                            bbq[:, o + r:o + r + 1])
                    fscs[br] = fsc
                    biasq = stats_pool.tile([128, nb * NHC], F32, tag="biasq",
                                            bufs=3, name=f"biasq_{br}_{q}")
                    for r in range(nb):
                        k = k0 + r
                        nc.vector.scalar_tensor_tensor(
                            biasq[:, r * NHC:(r + 1) * NHC],
                            ns1gt[:, k * NHC:(k + 1) * NHC],
                            bbq[:, o + nb + r:o + nb + r + 1],
                            b1pt[:, k * NHC:(k + 1) * NHC],
                            op0=ALU.mult, op1=ALU.add)
                    biasqs[br] = biasq
                return fscs, biasqs

            # ---- fused per-quad stats + band pipeline ----
            for q, (k0, nb) in enumerate(QUADS):
                fq = {}
                wq = {}
                for br in ("m", "p"):
                    if q == 0:
                        fq[br], wq[br] = fq0[br], wq0[br]
                    else:
                        # split dispatch: m on sync, p on the scalar-engine
                        # DGE so the four per-quad fetches don't serialize
                        # on one initiator queue (feat_p otherwise lands
                        # ~12us into the quad and stalls the next chain)
                        eng = nc.sync if br == "m" else nc.scalar
                        fq[br] = featk_pool.tile([128, nb * T], FP16,
                                                 tag="featq",
                                                 name=f"featq_{br}_{q}")
                        eng.dma_start(
                            fq[br][:], ins[f"feat_{br}"][:, k0 * T:(k0 + nb) * T])
                        wq[br] = w1t_pool.tile([128, nb * H], FP16, tag="w1q",
                                               name=f"w1q_{br}_{q}")
                        eng.dma_start(
                            wq[br][:], ins[f"w1gt_{br}"][:, k0 * H:(k0 + nb) * H])

                if q == 0:
                    fscs, biasqs = {}, {}
                    for br in ("m", "p"):
                        fs, bq = stats_chain(q, k0, nb, fq, (br,))
                        fscs.update(fs)
                        biasqs.update(bq)
                    # second warm-up burst: bridges the PE gap between the
                    # head warm-up and the first real fc1 so the clock
                    # doesn't fall back to the mid p-state
                    for wi in range(6):
                        wps = main_ps.tile([128, T], F32, tag="h1ps", bufs=5,
                                           name=f"warm2_{wi}")
                        nc.tensor.matmul(wps[:], warm_sb[:, 0:128],
                                         warm_sb[:], start=True, stop=True)
                else:
                    fscs, biasqs = stats_chain(q, k0, nb, fq, ("m", "p"))

                if q == 2:
                    nc.gpsimd.dma_start(halfpi[:], halfpi_d[:])
                    for br in ("m", "p"):
                        for j in range(2):
                            nc.gpsimd.dma_start(
                                noisy[br][:, j * T:(j + 1) * T],
                                ins[f"noisy_{br}"][j * 128:(j + 1) * 128, :])

                for br in ("m", "p"):
                    b1pt, ns1gt, w2tp, b2c = cb[br]
                    biasq = biasqs[br]
                    fsc = fscs[br]
                    h1s = []
                    for r in range(nb):
                        k = k0 + r
                        h1sb = h1sb_pool.tile([128, NHC * T], FP16, bufs=6)
                        h1s.append(h1sb)
                        for hc in range(NHC):
                            h1ps = main_ps.tile([128, T], F32, tag="h1ps",
                                                bufs=5,
                                                name=f"h1ps_{br}_{k}_{hc}")
                            nc.tensor.matmul(
                                h1ps[:],
                                wq[br][:, (r * NHC + hc) * 128:
                                       (r * NHC + hc + 1) * 128],
                                fsc[:, r * T:(r + 1) * T],
                                start=True, stop=True)
                            nc.scalar.activation(
                                h1sb[:, hc * T:(hc + 1) * T], h1ps[:],
                                AF.Tanh,
                                bias=biasq[:, r * NHC + hc:r * NHC + hc + 1])
                    fc2g = main_ps.tile([128, T], F32, tag="fc2ps", bufs=2,
                                        name=f"fc2g_{br}_{q}")
                    for r in range(nb):
                        k = k0 + r
                        wp, woff = WPADS[k], int(WOFFS[k])
                        for hc in range(NHC):
                            nc.tensor.matmul(
                                fc2g[32 * r:32 * r + wp, :],
                                w2tp[:, hc * WPTOT + woff:
                                     hc * WPTOT + woff + wp],
                                h1s[r][:, hc * T:(hc + 1) * T],
                                start=(hc == 0), stop=(hc == NHC - 1),
                                tile_position=(0, 32 * r))
                    grp_t = band_pool.tile([128, T], FP16, tag="band")
                    nc.scalar.activation(grp_t[:], fc2g[:], AF.Tanh,
                                         bias=b2c[:, q:q + 1])
                    for r in range(nb):
                        k = k0 + r
                        w, off = BANDS[k], int(OFFS[k])
                        j0, r0 = off // 128, off % 128
                        if off + w <= (j0 + 1) * 128:
                            nc.gpsimd.dma_start(
                                masks[br][r0:r0 + w, j0 * T:(j0 + 1) * T],
                                grp_t[32 * r:32 * r + w, :])
                        else:
                            n1 = (j0 + 1) * 128 - off
                            nc.gpsimd.dma_start(
                                masks[br][r0:128, j0 * T:(j0 + 1) * T],
                                grp_t[32 * r:32 * r + n1, :])
                            rem = w - n1
                            if j0 + 1 < 2:
                                nc.gpsimd.dma_start(
                                    masks[br][0:rem, (j0 + 1) * T:(j0 + 2) * T],
                                    grp_t[32 * r + n1:32 * r + w, :])
                            else:
                                nc.gpsimd.dma_start(
                                    masks[br + "2"][0:rem, :],
                                    grp_t[32 * r + n1:32 * r + w, :])

            # ---- batched finale over both f-chunks [128, 2T] ----
            W2T = 2 * T
            y_m = masks["m"][:]
            y_p = masks["p"][:]
            ang = fin_pool.tile([128, W2T], FP16, tag="ang", name="ang")
            nc.vector.scalar_tensor_tensor(ang[:], y_p, PI, noisy["p"][:],
                                           op0=ALU.mult, op1=ALU.add)
            t2 = fin_pool.tile([128, W2T], FP16, tag="t2", name="t2")
            nc.vector.tensor_scalar(t2[:], ang[:], INV2PI, MAGIC16,
                                    op0=ALU.mult, op1=ALU.add)
            m2pin = fin_pool.tile([128, W2T], FP16, tag="m2pin", name="m2pin")
            nc.vector.tensor_scalar(m2pin[:], t2[:], MAGIC16, N2PI,
                                    op0=ALU.subtract, op1=ALU.mult)
            nc.vector.tensor_add(m2pin[:], ang[:], m2pin[:])
            sn = fin_pool.tile([128, W2T], FP16, tag="sn", name="sn")
            nc.scalar.activation(sn[:], m2pin[:], AF.Sin)
            t2c = fin_pool.tile([128, W2T], FP16, tag="t2c", name="t2c")
            nc.vector.tensor_scalar(t2c[:], ang[:], INV2PI, 0.25,
                                    op0=ALU.mult, op1=ALU.add)
            nc.vector.tensor_scalar_add(t2c[:], t2c[:], MAGIC16)
            m2pinc = fin_pool.tile([128, W2T], FP16, tag="m2pinc",
                                   name="m2pinc")
            nc.vector.tensor_scalar(m2pinc[:], t2c[:], MAGIC16, N2PI,
                                    op0=ALU.subtract, op1=ALU.mult)
            nc.vector.tensor_add(m2pinc[:], ang[:], m2pinc[:])
            cn = fin_pool.tile([128, W2T], FP16, tag="cn", name="cn")
            nc.scalar.activation(cn[:], m2pinc[:], AF.Sin, bias=halfpi[:])
            mk = fin_pool.tile([128, W2T], FP16, tag="mk", name="mk")
            nc.vector.tensor_scalar(mk[:], y_m, 0.5, 0.5,
                                    op0=ALU.mult, op1=ALU.add)
            enh = fin_pool.tile([128, W2T], FP16, tag="enh", name="enh")
            nc.vector.tensor_mul(enh[:], mk[:], noisy["m"][:])
            ot = fin_pool.tile([128, 2 * W2T], FP16, tag="ot", name="ot")
            ot4 = ot[:].rearrange("p (j two t) -> p j two t", j=2, two=2)
            enh2 = enh[:].rearrange("p (j t) -> p j t", j=2)
            cn2 = cn[:].rearrange("p (j t) -> p j t", j=2)
            sn2 = sn[:].rearrange("p (j t) -> p j t", j=2)
            nc.vector.tensor_mul(ot4[:, :, 0, :], enh2, cn2)
            nc.vector.tensor_mul(ot4[:, :, 1, :], enh2, sn2)
            for j in range(2):
                nc.sync.dma_start(out_d[j * 128:(j + 1) * 128, :],
                                  ot[:, j * W2T:(j + 1) * W2T])
            nc.sync.dma_start(aux_d[0:1, :], masks["m2"][:])
            nc.sync.dma_start(aux_d[1:2, :], masks["p2"][:])

    nc.compile()
    return nc


def kernel(mag_features, phase_features, noisy_mag, noisy_phase,
           mag_gamma, mag_beta, mag_W1, mag_b1, mag_W2, mag_b2,
           ph_gamma, ph_beta, ph_W1, ph_b1, ph_W2, ph_b2):
    if "nc" not in _cache:
        _cache["nc"] = _build()
    nc = _cache["nc"]

    mW1gT, mb1pT, mns1gt, mW2Tp, mb2c = _prep_branch(
        np.asarray(mag_gamma), np.asarray(mag_beta), np.asarray(mag_W1),
        np.asarray(mag_b1), np.asarray(mag_W2), np.asarray(mag_b2),
        is_mag=True)
    pW1gT, pb1pT, pns1gt, pW2Tp, pb2c = _prep_branch(
        np.asarray(ph_gamma), np.asarray(ph_beta), np.asarray(ph_W1),
        np.asarray(ph_b1), np.asarray(ph_W2), np.asarray(ph_b2),
        is_mag=False)

    shared = dict(
        w1gt_m=mW1gT, b1pt_m=mb1pT, ns1gt_m=mns1gt, w2tp_m=mW2Tp, b2c_m=mb2c,
        w1gt_p=pW1gT, b1pt_p=pb1pT, ns1gt_p=pns1gt, w2tp_p=pW2Tp, b2c_p=pb2c,
        halfpi=np.full((128, 1), np.pi / 2, np.float32),
    )
    mag_features = np.asarray(mag_features)
    phase_features = np.asarray(phase_features)
    noisy_mag = np.asarray(noisy_mag, dtype=np.float32)
    noisy_phase = np.asarray(noisy_phase, dtype=np.float32)

    in_maps = []
    for b in range(B):
        m = dict(shared)
        # [C, T, K] -> [C, K, T] k-major, contiguous per-band slices; fp16
        m["feat_m"] = np.ascontiguousarray(
            mag_features[b].transpose(0, 2, 1)).reshape(C, K * T).astype(
                np.float16)
        m["feat_p"] = np.ascontiguousarray(
            phase_features[b].transpose(0, 2, 1)).reshape(C, K * T).astype(
                np.float16)
        m["noisy_m"] = np.ascontiguousarray(noisy_mag[b][0:256]).astype(
            np.float16)
        m["noisy_p"] = np.ascontiguousarray(noisy_phase[b][0:256]).astype(
            np.float16)
        in_maps.append(m)

    import os
    trace = bool(os.environ.get("BASS_PROFILE"))
    res = run_bass_kernel_spmd(nc, in_maps, list(range(B)), trace=trace)
    _cache["last_result"] = res

    out = np.empty((B, F, T), np.complex64)
    for b in range(B):
        o = res.results[b]["out"].astype(np.float32)   # [256, 2T] fp16
        out[b, 0:256].real = o[:, 0:T]
        out[b, 0:256].imag = o[:, T:2 * T]
        aux = res.results[b]["aux"].astype(np.float32)  # [2, T] fp16
        y_m, y_p = aux[0], aux[1]
        mask = 0.5 + 0.5 * y_m
        enh = noisy_mag[b, 256] * mask
        ang = noisy_phase[b, 256] + np.float32(np.pi) * y_p
        out[b, 256] = (enh * np.exp(1j * ang)).astype(np.complex64)
    return out


# revision 37
# speedup vs baseline: 1.1458x; 1.1458x over previous
"""Trainium2 Bass kernel for nn_DualBranchDecoder.

Dual-branch band-split decoder: per-band GroupNorm -> fc1(C=128->H=512)+tanh
-> per-band fc2(H->w_k) -> sigmoid mag mask / tanh phase offset -> complex out.

Sharding: data-parallel over batch B=8 across 8 NeuronCores (one sample per
core).

Design notes:
- feat host-cast to fp16. GroupNorm folded: feat is pre-scaled by inv on DVE
  (fp16 4x mode), the mean term goes into the fc1 activation bias
  (b1p - inv*mu*S1g), so fc1 tanh needs only a bias AP (a scale AP costs
  ~83ns/activation extra on the scalar engine).
- sigmoid rewritten as 0.5 + 0.5*tanh(x/2): the whole kernel runs on the
  tanh table until the finale's Sin; the affine folds into the finale.
- per-quad stats (bn_stats -> cross-partition reduce -> quake-rsqrt Newton)
  batched across both branches; quad 0 runs per-branch chains to shorten
  the head.
- finale: one batched block over [128, 2T] fp16 at the end; fp16 outputs
  (host upcasts). Freq row 256 assembled on host from the aux mask rows.
"""
import sys
sys.path.insert(0, '/opt/trn_rl_repo')

import numpy as np
import ml_dtypes

import concourse.bacc as bacc
import concourse.tile as tile
import concourse.mybir as mybir
from concourse.bass_utils import run_bass_kernel_spmd

F32 = mybir.dt.float32
FP16 = mybir.dt.float16
I32 = mybir.dt.int32
AF = mybir.ActivationFunctionType
ALU = mybir.AluOpType

# problem constants (hardcoded per contract)
B, C, T = 8, 128, 512
BANDS = [2] + [3] * 10 + [8] * 12 + [16] * 7 + [17]
K = len(BANDS)                      # 31
F = sum(BANDS)                      # 257
H = 4 * C                           # 512
NHC = H // 128                      # 4 h-chunks
EPS = 1e-5

OFFS = np.concatenate([[0], np.cumsum(BANDS)]).astype(int)   # band start freqs
WPADS = [w + (w & 1) for w in BANDS]                         # even-M pad
WOFFS = np.concatenate([[0], np.cumsum(WPADS)]).astype(int)
WPTOT = int(WOFFS[-1])

QUADS = [(4 * i, 4) for i in range(7)] + [(28, 3)]
NQ = len(QUADS)
MAGIC16 = 1536.0                     # 1.5 * 2**10: fp16 round-to-int magic
INV2PI = float(1.0 / (2 * np.pi))
N2PI = float(-2 * np.pi)
PI = float(np.pi)
N_WARM = 6

_cache = {}


def _prep_branch(gamma, beta, W1, b1, W2, b2, is_mag):
    """Host-side constant prep for one branch."""
    # W1gT[c, k*H + h] = W1[k,h,c] * gamma[k,c]
    W1g = W1 * gamma[:, None, :]                      # [K, H, C]
    W1gT = np.ascontiguousarray(W1g.transpose(2, 0, 1).reshape(C, K * H))
    W1gT16 = W1gT.astype(np.float16)
    # b1p[k,h] = b1[k,h] + sum_c W1[k,h,c]*beta[k,c];  layout [128, K*NHC]
    b1p = b1 + np.einsum('khc,kc->kh', W1, beta)      # [K, H]
    b1pT = np.zeros((128, K * NHC), np.float32)
    # ns1gt[p, k*NHC+hc] = -sum_c W1g16[c, k, hc*128+p]
    ns1gt = np.zeros((128, K * NHC), np.float32)
    W1gT16f = W1gT16.astype(np.float32)
    for k in range(K):
        for hc in range(NHC):
            b1pT[:, k * NHC + hc] = b1p[k, hc * 128:(hc + 1) * 128]
            ns1gt[:, k * NHC + hc] = -W1gT16f[:, k * H + hc * 128:
                                              k * H + (hc + 1) * 128].sum(axis=0)
    # mag branch computes y = tanh(0.5*(fc2 + b2)): fold 0.5 into W2 and b2
    wscale = 0.5 if is_mag else 1.0
    W2Tp = np.zeros((128, NHC * WPTOT), np.float32)
    for k in range(K):
        w, off, woff = BANDS[k], OFFS[k], WOFFS[k]
        for hc in range(NHC):
            W2Tp[:, hc * WPTOT + woff: hc * WPTOT + woff + w] = \
                wscale * W2[off:off + w, hc * 128:(hc + 1) * 128].T
    W2Tp = W2Tp.astype(np.float16)
    b2g = np.zeros((128, len(QUADS)), np.float32)
    for q, (k0, nb) in enumerate(QUADS):
        for r in range(nb):
            k = k0 + r
            b2g[32 * r:32 * r + BANDS[k], q] = \
                wscale * b2[OFFS[k]:OFFS[k] + BANDS[k]]
    return W1gT16, b1pT, ns1gt, W2Tp, b2g


def _build():
    nc = bacc.Bacc("TRN2", target_bir_lowering=False)

    # per-core inputs
    ins = {}
    for br in ("m", "p"):
        ins[f"feat_{br}"] = nc.dram_tensor(f"feat_{br}", [C, K * T], FP16,
                                           kind="ExternalInput")
        ins[f"w1gt_{br}"] = nc.dram_tensor(f"w1gt_{br}", [C, K * H], FP16,
                                           kind="ExternalInput")
        ins[f"b1pt_{br}"] = nc.dram_tensor(f"b1pt_{br}", [128, K * NHC], F32,
                                           kind="ExternalInput")
        ins[f"ns1gt_{br}"] = nc.dram_tensor(f"ns1gt_{br}", [128, K * NHC], F32,
                                            kind="ExternalInput")
        ins[f"w2tp_{br}"] = nc.dram_tensor(f"w2tp_{br}", [128, NHC * WPTOT],
                                           FP16, kind="ExternalInput")
        ins[f"b2c_{br}"] = nc.dram_tensor(f"b2c_{br}", [128, len(QUADS)], F32,
                                          kind="ExternalInput")
        ins[f"noisy_{br}"] = nc.dram_tensor(f"noisy_{br}", [2 * 128, T], FP16,
                                            kind="ExternalInput")
    halfpi_d = nc.dram_tensor("halfpi", [128, 1], F32, kind="ExternalInput")
    out_d = nc.dram_tensor("out", [2 * 128, 2 * T], FP16,
                           kind="ExternalOutput")
    aux_d = nc.dram_tensor("aux", [2, T], FP16, kind="ExternalOutput")

    with tile.TileContext(nc) as tc:
        with (
            tc.tile_pool(name="featk", bufs=3) as featk_pool,
            tc.tile_pool(name="fsc", bufs=3) as fsc_pool,
            tc.tile_pool(name="w1t", bufs=3) as w1t_pool,
            tc.tile_pool(name="h1sb", bufs=3) as h1sb_pool,
            tc.tile_pool(name="band", bufs=4) as band_pool,
            tc.tile_pool(name="const", bufs=1) as const_pool,
            tc.tile_pool(name="statsb", bufs=2) as stats_pool,
            tc.tile_pool(name="fin", bufs=1) as fin_pool,
            tc.tile_pool(name="mainps", bufs=1, space="PSUM") as main_ps,
        ):
            # ---- critical-path first: quad-0 fetches before anything ----
            k0_0, nb_0 = QUADS[0]
            eng_f = {"m": nc.sync, "p": nc.gpsimd}
            fq0 = {}
            wq0 = {}
            for br in ("m", "p"):
                fq0[br] = featk_pool.tile([128, nb_0 * T], FP16, tag="featq",
                                          name=f"featq_{br}_0")
                # per-band DMAs so band-0 stats can start early
                for r in range(nb_0):
                    eng_f[br].dma_start(
                        fq0[br][:, r * T:(r + 1) * T],
                        ins[f"feat_{br}"][:, (k0_0 + r) * T:(k0_0 + r + 1) * T])
                wq0[br] = w1t_pool.tile([128, nb_0 * H], FP16, tag="w1q",
                                        name=f"w1q_{br}_0")
                nc.scalar.dma_start(
                    wq0[br][:], ins[f"w1gt_{br}"][:, k0_0 * H:(k0_0 + nb_0) * H])

            # ---- all-ones stationary for the reduce-broadcast matmul ----
            # fp16 (with fp16 sums) keeps the per-quad ps_r matmul off the
            # slow f32 weight-load path (~600ns -> ~150ns)
            ones128 = const_pool.tile([128, 128], FP16, tag="ones128",
                                      name="ones128")
            nc.vector.memset(ones128[:], 1.0)
            # ---- PE warm-up from a memset tile (no DMA dependency) ----
            warm_sb = const_pool.tile([128, T], FP16, tag="warm_sb",
                                      name="warm_sb")
            nc.vector.memset(warm_sb[:], 0.001)
            for wi in range(N_WARM):
                wps = main_ps.tile([128, T], F32, tag="h1ps", bufs=5,
                                   name=f"warm_{wi}")
                nc.tensor.matmul(wps[:], warm_sb[:, 0:128], warm_sb[:],
                                 start=True, stop=True)

            cb = {}
            noisy = {}
            for br in ("m", "p"):
                b1pt = const_pool.tile([128, K * NHC], F32, tag=f"b1pt_{br}",
                                       name=f"b1pt_{br}")
                nc.scalar.dma_start(b1pt[:], ins[f"b1pt_{br}"][:])
                ns1gt = const_pool.tile([128, K * NHC], F32, tag=f"ns1gt_{br}",
                                        name=f"ns1gt_{br}")
                nc.scalar.dma_start(ns1gt[:], ins[f"ns1gt_{br}"][:])
                w2tp = const_pool.tile([128, NHC * WPTOT], FP16,
                                       tag=f"w2tp_{br}", name=f"w2tp_{br}")
                nc.gpsimd.dma_start(w2tp[:], ins[f"w2tp_{br}"][:])
                b2c = const_pool.tile([128, len(QUADS)], F32, tag=f"b2c_{br}",
                                      name=f"b2c_{br}")
                nc.sync.dma_start(b2c[:], ins[f"b2c_{br}"][:])
                cb[br] = (b1pt, ns1gt, w2tp, b2c)
                nz = const_pool.tile([128, 2 * T], FP16, tag=f"noisy_{br}",
                                     name=f"noisy_{br}")
                noisy[br] = nz
            halfpi = const_pool.tile([128, 1], F32)

            # ---- masks (tanh outputs y), fp16 ----
            masks = {}
            for br in ("m", "p"):
                masks[br] = const_pool.tile([128, 2 * T], FP16,
                                            tag=f"mask_{br}", name=f"mask_{br}")
                masks[br + "2"] = const_pool.tile([1, T], FP16,
                                                  tag=f"mask2_{br}",
                                                  name=f"mask2_{br}")

            def stats_chain(q, k0, nb, fq, branches):
                """bn stats -> reduce-broadcast -> newton rsqrt -> prescaled
                feat + per-(band,hc) activation bias for `branches`.

                Returns ({br: fsc}, {br: biasq}).
                """
                nbr = len(branches)
                sfx = f"{q}_{branches[0]}"
                sums = stats_pool.tile([128, 2 * nb * nbr], FP16, tag="sums",
                                       bufs=3, name=f"sums_{sfx}")
                for bi, br in enumerate(branches):
                    o = 2 * nb * bi
                    st_q = stats_pool.tile([128, nb * 6], F32, tag="st_q",
                                           name=f"st_{br}_{q}")
                    ag_q = stats_pool.tile([128, nb * 2], F32, tag="ag_q",
                                           name=f"ag_{br}_{q}")
                    for r in range(nb):
                        nc.vector.bn_stats(st_q[:, r * 6:(r + 1) * 6],
                                           fq[br][:, r * T:(r + 1) * T])
                        nc.vector.bn_aggr(ag_q[:, r * 2:(r + 1) * 2],
                                          st_q[:, r * 6:(r + 1) * 6])
                    ag3 = ag_q[:].rearrange("c (k two) -> c k two", two=2)
                    mean_ap = ag3[:, :, 0]
                    var_ap = ag3[:, :, 1]
                    nc.vector.tensor_copy(sums[:, o:o + nb], mean_ap)
                    tmp = stats_pool.tile([128, nb], F32, tag="tmp",
                                          name=f"tmp_{br}_{q}")
                    nc.vector.tensor_mul(tmp[:], mean_ap, mean_ap)
                    nc.vector.tensor_add(sums[:, o + nb:o + 2 * nb], tmp[:],
                                         var_ap)
                # reduce + broadcast in one matmul: out[m, j] = sum_p sums[p,j]
                ps_r = main_ps.tile([128, 2 * nb * nbr], F32, tag="ps_s",
                                    bufs=1, name=f"ps_r_{sfx}")
                nc.tensor.matmul(ps_r[:], ones128[:], sums[:],
                                 start=True, stop=True)
                g = stats_pool.tile([128, 2 * nb * nbr], F32, tag="g",
                                    name=f"g_{sfx}")
                nc.vector.tensor_scalar_mul(g[:], ps_r[:], 1.0 / C)
                g4 = g[:].rearrange("o (b two n) -> o b two n", b=nbr, two=2)
                gmean = g4[:, :, 0, :]
                gsq = g4[:, :, 1, :]
                gm2 = stats_pool.tile([128, nb * nbr], F32, tag="gm2",
                                      name=f"gm2_{sfx}")
                nc.vector.tensor_mul(gm2[:], gmean, gmean)
                vv = stats_pool.tile([128, nb * nbr], F32, tag="vv",
                                     name=f"vv_{sfx}")
                nc.vector.tensor_sub(vv[:], gsq, gm2[:])
                nc.vector.tensor_scalar_add(vv[:], vv[:], EPS)
                yy = stats_pool.tile([128, nb * nbr], F32, tag="yy",
                                     name=f"yy_{sfx}")
                nc.vector.tensor_scalar(yy[:].bitcast(I32), vv[:].bitcast(I32),
                                        1, -1, op0=ALU.arith_shift_right,
                                        op1=ALU.bitwise_xor)
                nc.vector.tensor_scalar_add(yy[:].bitcast(I32),
                                            yy[:].bitcast(I32), 0x5f3759e0)
                bbq = stats_pool.tile([128, 2 * nb * nbr], F32, tag="bbq",
                                      bufs=3, name=f"bbq_{sfx}")
                iv4 = bbq[:].rearrange("o (b two n) -> o b two n",
                                       b=nbr, two=2)
                inv_ap = iv4[:, :, 0, :]
                invmu_ap = iv4[:, :, 1, :]
                tnr = stats_pool.tile([128, nb * nbr], F32, tag="tnr",
                                      name=f"tnr_{sfx}")
                for it in range(3):
                    nc.vector.tensor_mul(tnr[:], yy[:], yy[:])
                    nc.vector.tensor_mul(tnr[:], tnr[:], vv[:])
                    nc.vector.tensor_scalar(tnr[:], tnr[:], -0.5, 1.5,
                                            op0=ALU.mult, op1=ALU.add)
                    dst = yy[:] if it < 2 else inv_ap
                    nc.vector.tensor_mul(dst, yy[:], tnr[:])
                nc.vector.tensor_mul(invmu_ap, inv_ap, gmean)

                fscs = {}
                biasqs = {}
                for bi, br in enumerate(branches):
                    o = 2 * nb * bi
                    b1pt, ns1gt, _, _ = cb[br]
                    fsc = fsc_pool.tile([128, nb * T], FP16, tag="fsc",
                                        name=f"fsc_{br}_{q}")
                    for r in range(nb):
                        nc.vector.tensor_scalar_mul(
                            fsc[:, r * T:(r + 1) * T],
                            fq[br][:, r * T:(r + 1) * T],
                            bbq[:, o + r:o + r + 1])
                    fscs[br] = fsc
                    biasq = stats_pool.tile([128, nb * NHC], F32, tag="biasq",
                                            bufs=3, name=f"biasq_{br}_{q}")
                    for r in range(nb):
                        k = k0 + r
                        nc.vector.scalar_tensor_tensor(
                            biasq[:, r * NHC:(r + 1) * NHC],
                            ns1gt[:, k * NHC:(k + 1) * NHC],
                            bbq[:, o + nb + r:o + nb + r + 1],
                            b1pt[:, k * NHC:(k + 1) * NHC],
                            op0=ALU.mult, op1=ALU.add)
                    biasqs[br] = biasq
                return fscs, biasqs

            # ---- fused per-quad stats + band pipeline ----
            for q, (k0, nb) in enumerate(QUADS):
                fq = {}
                wq = {}
                for br in ("m", "p"):
                    if q == 0:
                        fq[br], wq[br] = fq0[br], wq0[br]
                    else:
                        fq[br] = featk_pool.tile([128, nb * T], FP16,
                                                 tag="featq",
                                                 name=f"featq_{br}_{q}")
                        nc.sync.dma_start(
                            fq[br][:], ins[f"feat_{br}"][:, k0 * T:(k0 + nb) * T])
                        wq[br] = w1t_pool.tile([128, nb * H], FP16, tag="w1q",
                                               name=f"w1q_{br}_{q}")
                        nc.sync.dma_start(
                            wq[br][:], ins[f"w1gt_{br}"][:, k0 * H:(k0 + nb) * H])

                if q == 0:
                    fscs, biasqs = {}, {}
                    for br in ("m", "p"):
                        fs, bq = stats_chain(q, k0, nb, fq, (br,))
                        fscs.update(fs)
                        biasqs.update(bq)
                    # second warm-up burst: bridges the PE gap between the
                    # head warm-up and the first real fc1 so the clock
                    # doesn't fall back to the mid p-state
                    for wi in range(6):
                        wps = main_ps.tile([128, T], F32, tag="h1ps", bufs=5,
                                           name=f"warm2_{wi}")
                        nc.tensor.matmul(wps[:], warm_sb[:, 0:128],
                                         warm_sb[:], start=True, stop=True)
                else:
                    fscs, biasqs = stats_chain(q, k0, nb, fq, ("m", "p"))

                if q == 2:
                    nc.gpsimd.dma_start(halfpi[:], halfpi_d[:])
                    for br in ("m", "p"):
                        for j in range(2):
                            nc.gpsimd.dma_start(
                                noisy[br][:, j * T:(j + 1) * T],
                                ins[f"noisy_{br}"][j * 128:(j + 1) * 128, :])

                for br in ("m", "p"):
                    b1pt, ns1gt, w2tp, b2c = cb[br]
                    biasq = biasqs[br]
                    fsc = fscs[br]
                    h1s = []
                    for r in range(nb):
                        k = k0 + r
                        h1sb = h1sb_pool.tile([128, NHC * T], FP16, bufs=6)
                        h1s.append(h1sb)
                        for hc in range(NHC):
                            h1ps = main_ps.tile([128, T], F32, tag="h1ps",
                                                bufs=5,
                                                name=f"h1ps_{br}_{k}_{hc}")
                            nc.tensor.matmul(
                                h1ps[:],
                                wq[br][:, (r * NHC + hc) * 128:
                                       (r * NHC + hc + 1) * 128],
                                fsc[:, r * T:(r + 1) * T],
                                start=True, stop=True)
                            nc.scalar.activation(
                                h1sb[:, hc * T:(hc + 1) * T], h1ps[:],
                                AF.Tanh,
                                bias=biasq[:, r * NHC + hc:r * NHC + hc + 1])
                    fc2g = main_ps.tile([128, T], F32, tag="fc2ps", bufs=2,
                                        name=f"fc2g_{br}_{q}")
                    for r in range(nb):
                        k = k0 + r
                        wp, woff = WPADS[k], int(WOFFS[k])
                        for hc in range(NHC):
                            nc.tensor.matmul(
                                fc2g[32 * r:32 * r + wp, :],
                                w2tp[:, hc * WPTOT + woff:
                                     hc * WPTOT + woff + wp],
                                h1s[r][:, hc * T:(hc + 1) * T],
                                start=(hc == 0), stop=(hc == NHC - 1),
                                tile_position=(0, 32 * r))
                    grp_t = band_pool.tile([128, T], FP16, tag="band")
                    nc.scalar.activation(grp_t[:], fc2g[:], AF.Tanh,
                                         bias=b2c[:, q:q + 1])
                    for r in range(nb):
                        k = k0 + r
                        w, off = BANDS[k], int(OFFS[k])
                        j0, r0 = off // 128, off % 128
                        if off + w <= (j0 + 1) * 128:
                            nc.gpsimd.dma_start(
                                masks[br][r0:r0 + w, j0 * T:(j0 + 1) * T],
                                grp_t[32 * r:32 * r + w, :])
                        else:
                            n1 = (j0 + 1) * 128 - off
                            nc.gpsimd.dma_start(
                                masks[br][r0:128, j0 * T:(j0 + 1) * T],
                                grp_t[32 * r:32 * r + n1, :])
                            rem = w - n1
                            if j0 + 1 < 2:
                                nc.gpsimd.dma_start(
                                    masks[br][0:rem, (j0 + 1) * T:(j0 + 2) * T],
                                    grp_t[32 * r + n1:32 * r + w, :])
                            else:
                                nc.gpsimd.dma_start(
                                    masks[br + "2"][0:rem, :],
                                    grp_t[32 * r + n1:32 * r + w, :])

            # ---- batched finale over both f-chunks [128, 2T] ----
            W2T = 2 * T
            y_m = masks["m"][:]
            y_p = masks["p"][:]
            ang = fin_pool.tile([128, W2T], FP16, tag="ang", name="ang")
            nc.vector.scalar_tensor_tensor(ang[:], y_p, PI, noisy["p"][:],
                                           op0=ALU.mult, op1=ALU.add)
            t2 = fin_pool.tile([128, W2T], FP16, tag="t2", name="t2")
            nc.vector.tensor_scalar(t2[:], ang[:], INV2PI, MAGIC16,
                                    op0=ALU.mult, op1=ALU.add)
            m2pin = fin_pool.tile([128, W2T], FP16, tag="m2pin", name="m2pin")
            nc.vector.tensor_scalar(m2pin[:], t2[:], MAGIC16, N2PI,
                                    op0=ALU.subtract, op1=ALU.mult)
            nc.vector.tensor_add(m2pin[:], ang[:], m2pin[:])
            sn = fin_pool.tile([128, W2T], FP16, tag="sn", name="sn")
            nc.scalar.activation(sn[:], m2pin[:], AF.Sin)
            t2c = fin_pool.tile([128, W2T], FP16, tag="t2c", name="t2c")
            nc.vector.tensor_scalar(t2c[:], ang[:], INV2PI, 0.25,
                                    op0=ALU.mult, op1=ALU.add)
            nc.vector.tensor_scalar_add(t2c[:], t2c[:], MAGIC16)
            m2pinc = fin_pool.tile([128, W2T], FP16, tag="m2pinc",
                                   name="m2pinc")
            nc.vector.tensor_scalar(m2pinc[:], t2c[:], MAGIC16, N2PI,
                                    op0=ALU.subtract, op1=ALU.mult)
            nc.vector.tensor_add(m2pinc[:], ang[:], m2pinc[:])
            cn = fin_pool.tile([128, W2T], FP16, tag="cn", name="cn")
            nc.scalar.activation(cn[:], m2pinc[:], AF.Sin, bias=halfpi[:])
            mk = fin_pool.tile([128, W2T], FP16, tag="mk", name="mk")
            nc.vector.tensor_scalar(mk[:], y_m, 0.5, 0.5,
                                    op0=ALU.mult, op1=ALU.add)
            enh = fin_pool.tile([128, W2T], FP16, tag="enh", name="enh")
            nc.vector.tensor_mul(enh[:], mk[:], noisy["m"][:])
            ot = fin_pool.tile([128, 2 * W2T], FP16, tag="ot", name="ot")
            ot4 = ot[:].rearrange("p (j two t) -> p j two t", j=2, two=2)
            enh2 = enh[:].rearrange("p (j t) -> p j t", j=2)
            cn2 = cn[:].rearrange("p (j t) -> p j t", j=2)
            sn2 = sn[:].rearrange("p (j t) -> p j t", j=2)
            nc.vector.tensor_mul(ot4[:, :, 0, :], enh2, cn2)
            nc.vector.tensor_mul(ot4[:, :, 1, :], enh2, sn2)
            for j in range(2):
                nc.sync.dma_start(out_d[j * 128:(j + 1) * 128, :],
                                  ot[:, j * W2T:(j + 1) * W2T])
            nc.sync.dma_start(aux_d[0:1, :], masks["m2"][:])
            nc.sync.dma_start(aux_d[1:2, :], masks["p2"][:])

    nc.compile()
    return nc


def kernel(mag_features, phase_features, noisy_mag, noisy_phase,
           mag_gamma, mag_beta, mag_W1, mag_b1, mag_W2, mag_b2,
           ph_gamma, ph_beta, ph_W1, ph_b1, ph_W2, ph_b2):
    if "nc" not in _cache:
        _cache["nc"] = _build()
    nc = _cache["nc"]

    mW1gT, mb1pT, mns1gt, mW2Tp, mb2c = _prep_branch(
        np.asarray(mag_gamma), np.asarray(mag_beta), np.asarray(mag_W1),
        np.asarray(mag_b1), np.asarray(mag_W2), np.asarray(mag_b2),
        is_mag=True)
    pW1gT, pb1pT, pns1gt, pW2Tp, pb2c = _prep_branch(
        np.asarray(ph_gamma), np.asarray(ph_beta), np.asarray(ph_W1),
        np.asarray(ph_b1), np.asarray(ph_W2), np.asarray(ph_b2),
        is_mag=False)

    shared = dict(
        w1gt_m=mW1gT, b1pt_m=mb1pT, ns1gt_m=mns1gt, w2tp_m=mW2Tp, b2c_m=mb2c,
        w1gt_p=pW1gT, b1pt_p=pb1pT, ns1gt_p=pns1gt, w2tp_p=pW2Tp, b2c_p=pb2c,
        halfpi=np.full((128, 1), np.pi / 2, np.float32),
    )
    mag_features = np.asarray(mag_features)
    phase_features = np.asarray(phase_features)
    noisy_mag = np.asarray(noisy_mag, dtype=np.float32)
    noisy_phase = np.asarray(noisy_phase, dtype=np.float32)

    in_maps = []
    for b in range(B):
        m = dict(shared)
        # [C, T, K] -> [C, K, T] k-major, contiguous per-band slices; fp16
        m["feat_m"] = np.ascontiguousarray(
            mag_features[b].transpose(0, 2, 1)).reshape(C, K * T).astype(
                np.float16)
        m["feat_p"] = np.ascontiguousarray(
            phase_features[b].transpose(0, 2, 1)).reshape(C, K * T).astype(
                np.float16)
        m["noisy_m"] = np.ascontiguousarray(noisy_mag[b][0:256]).astype(
            np.float16)
        m["noisy_p"] = np.ascontiguousarray(noisy_phase[b][0:256]).astype(
            np.float16)
        in_maps.append(m)

    import os
    trace = bool(os.environ.get("BASS_PROFILE"))
    res = run_bass_kernel_spmd(nc, in_maps, list(range(B)), trace=trace)
    _cache["last_result"] = res

    out = np.empty((B, F, T), np.complex64)
    for b in range(B):
        o = res.results[b]["out"].astype(np.float32)   # [256, 2T] fp16
        out[b, 0:256].real = o[:, 0:T]
        out[b, 0:256].imag = o[:, T:2 * T]
        aux = res.results[b]["aux"].astype(np.float32)  # [2, T] fp16
        y_m, y_p = aux[0], aux[1]
        mask = 0.5 + 0.5 * y_m
        enh = noisy_mag[b, 256] * mask
        ang = noisy_phase[b, 256] + np.float32(np.pi) * y_p
        out[b, 256] = (enh * np.exp(1j * ang)).astype(np.complex64)
    return out


# revision 38
# speedup vs baseline: 1.1646x; 1.0164x over previous
"""Trainium2 Bass kernel for nn_DualBranchDecoder.

Dual-branch band-split decoder: per-band GroupNorm -> fc1(C=128->H=512)+tanh
-> per-band fc2(H->w_k) -> sigmoid mag mask / tanh phase offset -> complex out.

Sharding: data-parallel over batch B=8 across 8 NeuronCores (one sample per
core).

Design notes:
- feat host-cast to fp16. GroupNorm folded: feat is pre-scaled by inv on DVE
  (fp16 4x mode), the mean term goes into the fc1 activation bias
  (b1p - inv*mu*S1g), so fc1 tanh needs only a bias AP (a scale AP costs
  ~83ns/activation extra on the scalar engine).
- sigmoid rewritten as 0.5 + 0.5*tanh(x/2): the whole kernel runs on the
  tanh table until the finale's Sin; the affine folds into the finale.
- per-quad stats (bn_stats -> cross-partition reduce -> quake-rsqrt Newton)
  batched across both branches; quad 0 runs per-branch chains to shorten
  the head.
- finale: one batched block over [128, 2T] fp16 at the end; fp16 outputs
  (host upcasts). Freq row 256 assembled on host from the aux mask rows.
"""
import sys
sys.path.insert(0, '/opt/trn_rl_repo')

import numpy as np
import ml_dtypes

import concourse.bacc as bacc
import concourse.tile as tile
import concourse.mybir as mybir
from concourse.bass_utils import run_bass_kernel_spmd

F32 = mybir.dt.float32
FP16 = mybir.dt.float16
I32 = mybir.dt.int32
AF = mybir.ActivationFunctionType
ALU = mybir.AluOpType

# problem constants (hardcoded per contract)
B, C, T = 8, 128, 512
BANDS = [2] + [3] * 10 + [8] * 12 + [16] * 7 + [17]
K = len(BANDS)                      # 31
F = sum(BANDS)                      # 257
H = 4 * C                           # 512
NHC = H // 128                      # 4 h-chunks
EPS = 1e-5

OFFS = np.concatenate([[0], np.cumsum(BANDS)]).astype(int)   # band start freqs
WPADS = [w + (w & 1) for w in BANDS]                         # even-M pad
WOFFS = np.concatenate([[0], np.cumsum(WPADS)]).astype(int)
WPTOT = int(WOFFS[-1])

QUADS = [(4 * i, 4) for i in range(7)] + [(28, 3)]
NQ = len(QUADS)
MAGIC16 = 1536.0                     # 1.5 * 2**10: fp16 round-to-int magic
INV2PI = float(1.0 / (2 * np.pi))
N2PI = float(-2 * np.pi)
PI = float(np.pi)
N_WARM = 6

_cache = {}


def _prep_branch(gamma, beta, W1, b1, W2, b2, is_mag):
    """Host-side constant prep for one branch."""
    # W1gT[c, k*H + h] = W1[k,h,c] * gamma[k,c]
    W1g = W1 * gamma[:, None, :]                      # [K, H, C]
    W1gT = np.ascontiguousarray(W1g.transpose(2, 0, 1).reshape(C, K * H))
    W1gT16 = W1gT.astype(np.float16)
    # b1p[k,h] = b1[k,h] + sum_c W1[k,h,c]*beta[k,c];  layout [128, K*NHC]
    b1p = b1 + np.einsum('khc,kc->kh', W1, beta)      # [K, H]
    b1pT = np.zeros((128, K * NHC), np.float32)
    # ns1gt[p, k*NHC+hc] = -sum_c W1g16[c, k, hc*128+p]
    ns1gt = np.zeros((128, K * NHC), np.float32)
    W1gT16f = W1gT16.astype(np.float32)
    for k in range(K):
        for hc in range(NHC):
            b1pT[:, k * NHC + hc] = b1p[k, hc * 128:(hc + 1) * 128]
            ns1gt[:, k * NHC + hc] = -W1gT16f[:, k * H + hc * 128:
                                              k * H + (hc + 1) * 128].sum(axis=0)
    # mag branch computes y = tanh(0.5*(fc2 + b2)): fold 0.5 into W2 and b2
    wscale = 0.5 if is_mag else 1.0
    W2Tp = np.zeros((128, NHC * WPTOT), np.float32)
    for k in range(K):
        w, off, woff = BANDS[k], OFFS[k], WOFFS[k]
        for hc in range(NHC):
            W2Tp[:, hc * WPTOT + woff: hc * WPTOT + woff + w] = \
                wscale * W2[off:off + w, hc * 128:(hc + 1) * 128].T
    W2Tp = W2Tp.astype(np.float16)
    b2g = np.zeros((128, len(QUADS)), np.float32)
    for q, (k0, nb) in enumerate(QUADS):
        for r in range(nb):
            k = k0 + r
            b2g[32 * r:32 * r + BANDS[k], q] = \
                wscale * b2[OFFS[k]:OFFS[k] + BANDS[k]]
    return W1gT16, b1pT, ns1gt, W2Tp, b2g


def _build():
    nc = bacc.Bacc("TRN2", target_bir_lowering=False)

    # per-core inputs
    ins = {}
    for br in ("m", "p"):
        ins[f"feat_{br}"] = nc.dram_tensor(f"feat_{br}", [C, K * T], FP16,
                                           kind="ExternalInput")
        ins[f"w1gt_{br}"] = nc.dram_tensor(f"w1gt_{br}", [C, K * H], FP16,
                                           kind="ExternalInput")
        ins[f"b1pt_{br}"] = nc.dram_tensor(f"b1pt_{br}", [128, K * NHC], F32,
                                           kind="ExternalInput")
        ins[f"ns1gt_{br}"] = nc.dram_tensor(f"ns1gt_{br}", [128, K * NHC], F32,
                                            kind="ExternalInput")
        ins[f"w2tp_{br}"] = nc.dram_tensor(f"w2tp_{br}", [128, NHC * WPTOT],
                                           FP16, kind="ExternalInput")
        ins[f"b2c_{br}"] = nc.dram_tensor(f"b2c_{br}", [128, len(QUADS)], F32,
                                          kind="ExternalInput")
        ins[f"noisy_{br}"] = nc.dram_tensor(f"noisy_{br}", [2 * 128, T], FP16,
                                            kind="ExternalInput")
    halfpi_d = nc.dram_tensor("halfpi", [128, 1], F32, kind="ExternalInput")
    out_d = nc.dram_tensor("out", [2 * 128, 2 * T], FP16,
                           kind="ExternalOutput")
    aux_d = nc.dram_tensor("aux", [2, T], FP16, kind="ExternalOutput")

    with tile.TileContext(nc) as tc:
        with (
            tc.tile_pool(name="featk", bufs=4) as featk_pool,
            tc.tile_pool(name="fsc", bufs=3) as fsc_pool,
            tc.tile_pool(name="w1t", bufs=4) as w1t_pool,
            tc.tile_pool(name="h1sb", bufs=3) as h1sb_pool,
            tc.tile_pool(name="band", bufs=4) as band_pool,
            tc.tile_pool(name="const", bufs=1) as const_pool,
            tc.tile_pool(name="statsb", bufs=2) as stats_pool,
            tc.tile_pool(name="fin", bufs=1) as fin_pool,
            tc.tile_pool(name="mainps", bufs=1, space="PSUM") as main_ps,
        ):
            # ---- critical-path first: quad-0 fetches before anything ----
            k0_0, nb_0 = QUADS[0]
            eng_f = {"m": nc.sync, "p": nc.gpsimd}
            fq0 = {}
            wq0 = {}
            for br in ("m", "p"):
                fq0[br] = featk_pool.tile([128, nb_0 * T], FP16, tag="featq",
                                          name=f"featq_{br}_0")
                # per-band DMAs so band-0 stats can start early
                for r in range(nb_0):
                    eng_f[br].dma_start(
                        fq0[br][:, r * T:(r + 1) * T],
                        ins[f"feat_{br}"][:, (k0_0 + r) * T:(k0_0 + r + 1) * T])
                wq0[br] = w1t_pool.tile([128, nb_0 * H], FP16, tag="w1q",
                                        name=f"w1q_{br}_0")
                nc.scalar.dma_start(
                    wq0[br][:], ins[f"w1gt_{br}"][:, k0_0 * H:(k0_0 + nb_0) * H])

            # ---- all-ones stationary for the reduce-broadcast matmul ----
            # fp16 (with fp16 sums) keeps the per-quad ps_r matmul off the
            # slow f32 weight-load path (~600ns -> ~150ns)
            ones128 = const_pool.tile([128, 128], FP16, tag="ones128",
                                      name="ones128")
            nc.vector.memset(ones128[:], 1.0)
            # ---- PE warm-up from a memset tile (no DMA dependency) ----
            warm_sb = const_pool.tile([128, T], FP16, tag="warm_sb",
                                      name="warm_sb")
            nc.vector.memset(warm_sb[:], 0.001)
            for wi in range(N_WARM):
                wps = main_ps.tile([128, T], F32, tag="h1ps", bufs=5,
                                   name=f"warm_{wi}")
                nc.tensor.matmul(wps[:], warm_sb[:, 0:128], warm_sb[:],
                                 start=True, stop=True)

            cb = {}
            noisy = {}
            for br in ("m", "p"):
                b1pt = const_pool.tile([128, K * NHC], F32, tag=f"b1pt_{br}",
                                       name=f"b1pt_{br}")
                nc.scalar.dma_start(b1pt[:], ins[f"b1pt_{br}"][:])
                ns1gt = const_pool.tile([128, K * NHC], F32, tag=f"ns1gt_{br}",
                                        name=f"ns1gt_{br}")
                nc.scalar.dma_start(ns1gt[:], ins[f"ns1gt_{br}"][:])
                w2tp = const_pool.tile([128, NHC * WPTOT], FP16,
                                       tag=f"w2tp_{br}", name=f"w2tp_{br}")
                nc.gpsimd.dma_start(w2tp[:], ins[f"w2tp_{br}"][:])
                b2c = const_pool.tile([128, len(QUADS)], F32, tag=f"b2c_{br}",
                                      name=f"b2c_{br}")
                nc.sync.dma_start(b2c[:], ins[f"b2c_{br}"][:])
                cb[br] = (b1pt, ns1gt, w2tp, b2c)
                nz = const_pool.tile([128, 2 * T], FP16, tag=f"noisy_{br}",
                                     name=f"noisy_{br}")
                noisy[br] = nz
            halfpi = const_pool.tile([128, 1], F32)

            # ---- masks (tanh outputs y), fp16 ----
            masks = {}
            for br in ("m", "p"):
                masks[br] = const_pool.tile([128, 2 * T], FP16,
                                            tag=f"mask_{br}", name=f"mask_{br}")
                masks[br + "2"] = const_pool.tile([1, T], FP16,
                                                  tag=f"mask2_{br}",
                                                  name=f"mask2_{br}")

            def stats_chain(q, k0, nb, fq, branches):
                """bn stats -> reduce-broadcast -> newton rsqrt -> prescaled
                feat + per-(band,hc) activation bias for `branches`.

                Returns ({br: fsc}, {br: biasq}).
                """
                nbr = len(branches)
                sfx = f"{q}_{branches[0]}"
                sums = stats_pool.tile([128, 2 * nb * nbr], FP16, tag="sums",
                                       bufs=3, name=f"sums_{sfx}")
                for bi, br in enumerate(branches):
                    o = 2 * nb * bi
                    st_q = stats_pool.tile([128, nb * 6], F32, tag="st_q",
                                           name=f"st_{br}_{q}")
                    ag_q = stats_pool.tile([128, nb * 2], F32, tag="ag_q",
                                           name=f"ag_{br}_{q}")
                    for r in range(nb):
                        nc.vector.bn_stats(st_q[:, r * 6:(r + 1) * 6],
                                           fq[br][:, r * T:(r + 1) * T])
                        nc.vector.bn_aggr(ag_q[:, r * 2:(r + 1) * 2],
                                          st_q[:, r * 6:(r + 1) * 6])
                    ag3 = ag_q[:].rearrange("c (k two) -> c k two", two=2)
                    mean_ap = ag3[:, :, 0]
                    var_ap = ag3[:, :, 1]
                    nc.vector.tensor_copy(sums[:, o:o + nb], mean_ap)
                    tmp = stats_pool.tile([128, nb], F32, tag="tmp",
                                          name=f"tmp_{br}_{q}")
                    nc.vector.tensor_mul(tmp[:], mean_ap, mean_ap)
                    nc.vector.tensor_add(sums[:, o + nb:o + 2 * nb], tmp[:],
                                         var_ap)
                # reduce + broadcast in one matmul: out[m, j] = sum_p sums[p,j]
                ps_r = main_ps.tile([128, 2 * nb * nbr], F32, tag="ps_s",
                                    bufs=1, name=f"ps_r_{sfx}")
                nc.tensor.matmul(ps_r[:], ones128[:], sums[:],
                                 start=True, stop=True)
                g = stats_pool.tile([128, 2 * nb * nbr], F32, tag="g",
                                    name=f"g_{sfx}")
                nc.vector.tensor_scalar_mul(g[:], ps_r[:], 1.0 / C)
                g4 = g[:].rearrange("o (b two n) -> o b two n", b=nbr, two=2)
                gmean = g4[:, :, 0, :]
                gsq = g4[:, :, 1, :]
                gm2 = stats_pool.tile([128, nb * nbr], F32, tag="gm2",
                                      name=f"gm2_{sfx}")
                nc.vector.tensor_mul(gm2[:], gmean, gmean)
                vv = stats_pool.tile([128, nb * nbr], F32, tag="vv",
                                     name=f"vv_{sfx}")
                nc.vector.tensor_sub(vv[:], gsq, gm2[:])
                nc.vector.tensor_scalar_add(vv[:], vv[:], EPS)
                yy = stats_pool.tile([128, nb * nbr], F32, tag="yy",
                                     name=f"yy_{sfx}")
                nc.vector.tensor_scalar(yy[:].bitcast(I32), vv[:].bitcast(I32),
                                        1, -1, op0=ALU.arith_shift_right,
                                        op1=ALU.bitwise_xor)
                nc.vector.tensor_scalar_add(yy[:].bitcast(I32),
                                            yy[:].bitcast(I32), 0x5f3759e0)
                bbq = stats_pool.tile([128, 2 * nb * nbr], F32, tag="bbq",
                                      bufs=3, name=f"bbq_{sfx}")
                iv4 = bbq[:].rearrange("o (b two n) -> o b two n",
                                       b=nbr, two=2)
                inv_ap = iv4[:, :, 0, :]
                invmu_ap = iv4[:, :, 1, :]
                tnr = stats_pool.tile([128, nb * nbr], F32, tag="tnr",
                                      name=f"tnr_{sfx}")
                for it in range(3):
                    nc.vector.tensor_mul(tnr[:], yy[:], yy[:])
                    nc.vector.tensor_mul(tnr[:], tnr[:], vv[:])
                    nc.vector.tensor_scalar(tnr[:], tnr[:], -0.5, 1.5,
                                            op0=ALU.mult, op1=ALU.add)
                    dst = yy[:] if it < 2 else inv_ap
                    nc.vector.tensor_mul(dst, yy[:], tnr[:])
                nc.vector.tensor_mul(invmu_ap, inv_ap, gmean)

                fscs = {}
                biasqs = {}
                for bi, br in enumerate(branches):
                    o = 2 * nb * bi
                    b1pt, ns1gt, _, _ = cb[br]
                    fsc = fsc_pool.tile([128, nb * T], FP16, tag="fsc",
                                        name=f"fsc_{br}_{q}")
                    for r in range(nb):
                        nc.vector.tensor_scalar_mul(
                            fsc[:, r * T:(r + 1) * T],
                            fq[br][:, r * T:(r + 1) * T],
                            bbq[:, o + r:o + r + 1])
                    fscs[br] = fsc
                    biasq = stats_pool.tile([128, nb * NHC], F32, tag="biasq",
                                            bufs=3, name=f"biasq_{br}_{q}")
                    for r in range(nb):
                        k = k0 + r
                        nc.vector.scalar_tensor_tensor(
                            biasq[:, r * NHC:(r + 1) * NHC],
                            ns1gt[:, k * NHC:(k + 1) * NHC],
                            bbq[:, o + nb + r:o + nb + r + 1],
                            b1pt[:, k * NHC:(k + 1) * NHC],
                            op0=ALU.mult, op1=ALU.add)
                    biasqs[br] = biasq
                return fscs, biasqs

            # ---- fused per-quad stats + band pipeline ----
            for q, (k0, nb) in enumerate(QUADS):
                fq = {}
                wq = {}
                for br in ("m", "p"):
                    if q == 0:
                        fq[br], wq[br] = fq0[br], wq0[br]
                    else:
                        fq[br] = featk_pool.tile([128, nb * T], FP16,
                                                 tag="featq",
                                                 name=f"featq_{br}_{q}")
                        nc.sync.dma_start(
                            fq[br][:], ins[f"feat_{br}"][:, k0 * T:(k0 + nb) * T])
                        wq[br] = w1t_pool.tile([128, nb * H], FP16, tag="w1q",
                                               name=f"w1q_{br}_{q}")
                        nc.sync.dma_start(
                            wq[br][:], ins[f"w1gt_{br}"][:, k0 * H:(k0 + nb) * H])

                if q == 0:
                    fscs, biasqs = {}, {}
                    for br in ("m", "p"):
                        fs, bq = stats_chain(q, k0, nb, fq, (br,))
                        fscs.update(fs)
                        biasqs.update(bq)
                    # second warm-up burst: bridges the PE gap between the
                    # head warm-up and the first real fc1 so the clock
                    # doesn't fall back to the mid p-state
                    for wi in range(6):
                        wps = main_ps.tile([128, T], F32, tag="h1ps", bufs=5,
                                           name=f"warm2_{wi}")
                        nc.tensor.matmul(wps[:], warm_sb[:, 0:128],
                                         warm_sb[:], start=True, stop=True)
                else:
                    fscs, biasqs = stats_chain(q, k0, nb, fq, ("m", "p"))

                if q == 2:
                    nc.gpsimd.dma_start(halfpi[:], halfpi_d[:])
                    for br in ("m", "p"):
                        for j in range(2):
                            nc.gpsimd.dma_start(
                                noisy[br][:, j * T:(j + 1) * T],
                                ins[f"noisy_{br}"][j * 128:(j + 1) * 128, :])

                for br in ("m", "p"):
                    b1pt, ns1gt, w2tp, b2c = cb[br]
                    biasq = biasqs[br]
                    fsc = fscs[br]
                    h1s = []
                    for r in range(nb):
                        k = k0 + r
                        h1sb = h1sb_pool.tile([128, NHC * T], FP16, bufs=6)
                        h1s.append(h1sb)
                        for hc in range(NHC):
                            h1ps = main_ps.tile([128, T], F32, tag="h1ps",
                                                bufs=5,
                                                name=f"h1ps_{br}_{k}_{hc}")
                            nc.tensor.matmul(
                                h1ps[:],
                                wq[br][:, (r * NHC + hc) * 128:
                                       (r * NHC + hc + 1) * 128],
                                fsc[:, r * T:(r + 1) * T],
                                start=True, stop=True)
                            nc.scalar.activation(
                                h1sb[:, hc * T:(hc + 1) * T], h1ps[:],
                                AF.Tanh,
                                bias=biasq[:, r * NHC + hc:r * NHC + hc + 1])
                    fc2g = main_ps.tile([128, T], F32, tag="fc2ps", bufs=2,
                                        name=f"fc2g_{br}_{q}")
                    for r in range(nb):
                        k = k0 + r
                        wp, woff = WPADS[k], int(WOFFS[k])
                        for hc in range(NHC):
                            nc.tensor.matmul(
                                fc2g[32 * r:32 * r + wp, :],
                                w2tp[:, hc * WPTOT + woff:
                                     hc * WPTOT + woff + wp],
                                h1s[r][:, hc * T:(hc + 1) * T],
                                start=(hc == 0), stop=(hc == NHC - 1),
                                tile_position=(0, 32 * r))
                    grp_t = band_pool.tile([128, T], FP16, tag="band")
                    nc.scalar.activation(grp_t[:], fc2g[:], AF.Tanh,
                                         bias=b2c[:, q:q + 1])
                    for r in range(nb):
                        k = k0 + r
                        w, off = BANDS[k], int(OFFS[k])
                        j0, r0 = off // 128, off % 128
                        if off + w <= (j0 + 1) * 128:
                            nc.gpsimd.dma_start(
                                masks[br][r0:r0 + w, j0 * T:(j0 + 1) * T],
                                grp_t[32 * r:32 * r + w, :])
                        else:
                            n1 = (j0 + 1) * 128 - off
                            nc.gpsimd.dma_start(
                                masks[br][r0:128, j0 * T:(j0 + 1) * T],
                                grp_t[32 * r:32 * r + n1, :])
                            rem = w - n1
                            if j0 + 1 < 2:
                                nc.gpsimd.dma_start(
                                    masks[br][0:rem, (j0 + 1) * T:(j0 + 2) * T],
                                    grp_t[32 * r + n1:32 * r + w, :])
                            else:
                                nc.gpsimd.dma_start(
                                    masks[br + "2"][0:rem, :],
                                    grp_t[32 * r + n1:32 * r + w, :])

            # ---- batched finale over both f-chunks [128, 2T] ----
            W2T = 2 * T
            y_m = masks["m"][:]
            y_p = masks["p"][:]
            ang = fin_pool.tile([128, W2T], FP16, tag="ang", name="ang")
            nc.vector.scalar_tensor_tensor(ang[:], y_p, PI, noisy["p"][:],
                                           op0=ALU.mult, op1=ALU.add)
            t2 = fin_pool.tile([128, W2T], FP16, tag="t2", name="t2")
            nc.vector.tensor_scalar(t2[:], ang[:], INV2PI, MAGIC16,
                                    op0=ALU.mult, op1=ALU.add)
            m2pin = fin_pool.tile([128, W2T], FP16, tag="m2pin", name="m2pin")
            nc.vector.tensor_scalar(m2pin[:], t2[:], MAGIC16, N2PI,
                                    op0=ALU.subtract, op1=ALU.mult)
            nc.vector.tensor_add(m2pin[:], ang[:], m2pin[:])
            sn = fin_pool.tile([128, W2T], FP16, tag="sn", name="sn")
            nc.scalar.activation(sn[:], m2pin[:], AF.Sin)
            t2c = fin_pool.tile([128, W2T], FP16, tag="t2c", name="t2c")
            nc.vector.tensor_scalar(t2c[:], ang[:], INV2PI, 0.25,
                                    op0=ALU.mult, op1=ALU.add)
            nc.vector.tensor_scalar_add(t2c[:], t2c[:], MAGIC16)
            m2pinc = fin_pool.tile([128, W2T], FP16, tag="m2pinc",
                                   name="m2pinc")
            nc.vector.tensor_scalar(m2pinc[:], t2c[:], MAGIC16, N2PI,
                                    op0=ALU.subtract, op1=ALU.mult)
            nc.vector.tensor_add(m2pinc[:], ang[:], m2pinc[:])
            cn = fin_pool.tile([128, W2T], FP16, tag="cn", name="cn")
            nc.scalar.activation(cn[:], m2pinc[:], AF.Sin, bias=halfpi[:])
            mk = fin_pool.tile([128, W2T], FP16, tag="mk", name="mk")
            nc.vector.tensor_scalar(mk[:], y_m, 0.5, 0.5,
                                    op0=ALU.mult, op1=ALU.add)
            enh = fin_pool.tile([128, W2T], FP16, tag="enh", name="enh")
            nc.vector.tensor_mul(enh[:], mk[:], noisy["m"][:])
            ot = fin_pool.tile([128, 2 * W2T], FP16, tag="ot", name="ot")
            ot4 = ot[:].rearrange("p (j two t) -> p j two t", j=2, two=2)
            enh2 = enh[:].rearrange("p (j t) -> p j t", j=2)
            cn2 = cn[:].rearrange("p (j t) -> p j t", j=2)
            sn2 = sn[:].rearrange("p (j t) -> p j t", j=2)
            nc.vector.tensor_mul(ot4[:, :, 0, :], enh2, cn2)
            nc.vector.tensor_mul(ot4[:, :, 1, :], enh2, sn2)
            for j in range(2):
                nc.sync.dma_start(out_d[j * 128:(j + 1) * 128, :],
                                  ot[:, j * W2T:(j + 1) * W2T])
            nc.sync.dma_start(aux_d[0:1, :], masks["m2"][:])
            nc.sync.dma_start(aux_d[1:2, :], masks["p2"][:])

    nc.compile()
    return nc


def kernel(mag_features, phase_features, noisy_mag, noisy_phase,
           mag_gamma, mag_beta, mag_W1, mag_b1, mag_W2, mag_b2,
           ph_gamma, ph_beta, ph_W1, ph_b1, ph_W2, ph_b2):
    if "nc" not in _cache:
        _cache["nc"] = _build()
    nc = _cache["nc"]

    mW1gT, mb1pT, mns1gt, mW2Tp, mb2c = _prep_branch(
        np.asarray(mag_gamma), np.asarray(mag_beta), np.asarray(mag_W1),
        np.asarray(mag_b1), np.asarray(mag_W2), np.asarray(mag_b2),
        is_mag=True)
    pW1gT, pb1pT, pns1gt, pW2Tp, pb2c = _prep_branch(
        np.asarray(ph_gamma), np.asarray(ph_beta), np.asarray(ph_W1),
        np.asarray(ph_b1), np.asarray(ph_W2), np.asarray(ph_b2),
        is_mag=False)

    shared = dict(
        w1gt_m=mW1gT, b1pt_m=mb1pT, ns1gt_m=mns1gt, w2tp_m=mW2Tp, b2c_m=mb2c,
        w1gt_p=pW1gT, b1pt_p=pb1pT, ns1gt_p=pns1gt, w2tp_p=pW2Tp, b2c_p=pb2c,
        halfpi=np.full((128, 1), np.pi / 2, np.float32),
    )
    mag_features = np.asarray(mag_features)
    phase_features = np.asarray(phase_features)
    noisy_mag = np.asarray(noisy_mag, dtype=np.float32)
    noisy_phase = np.asarray(noisy_phase, dtype=np.float32)

    in_maps = []
    for b in range(B):
        m = dict(shared)
        # [C, T, K] -> [C, K, T] k-major, contiguous per-band slices; fp16
        m["feat_m"] = np.ascontiguousarray(
            mag_features[b].transpose(0, 2, 1)).reshape(C, K * T).astype(
                np.float16)
        m["feat_p"] = np.ascontiguousarray(
            phase_features[b].transpose(0, 2, 1)).reshape(C, K * T).astype(
                np.float16)
        m["noisy_m"] = np.ascontiguousarray(noisy_mag[b][0:256]).astype(
            np.float16)
        m["noisy_p"] = np.ascontiguousarray(noisy_phase[b][0:256]).astype(
            np.float16)
        in_maps.append(m)

    import os
    trace = bool(os.environ.get("BASS_PROFILE"))
    res = run_bass_kernel_spmd(nc, in_maps, list(range(B)), trace=trace)
    _cache["last_result"] = res

    out = np.empty((B, F, T), np.complex64)
    for b in range(B):
        o = res.results[b]["out"].astype(np.float32)   # [256, 2T] fp16
        out[b, 0:256].real = o[:, 0:T]
        out[b, 0:256].imag = o[:, T:2 * T]
        aux = res.results[b]["aux"].astype(np.float32)  # [2, T] fp16
        y_m, y_p = aux[0], aux[1]
        mask = 0.5 + 0.5 * y_m
        enh = noisy_mag[b, 256] * mask
        ang = noisy_phase[b, 256] + np.float32(np.pi) * y_p
        out[b, 256] = (enh * np.exp(1j * ang)).astype(np.complex64)
    return out


# revision 39
# speedup vs baseline: 1.1682x; 1.0032x over previous
"""Trainium2 Bass kernel for nn_DualBranchDecoder.

Dual-branch band-split decoder: per-band GroupNorm -> fc1(C=128->H=512)+tanh
-> per-band fc2(H->w_k) -> sigmoid mag mask / tanh phase offset -> complex out.

Sharding: data-parallel over batch B=8 across 8 NeuronCores (one sample per
core).

Design notes:
- feat host-cast to fp16. GroupNorm folded: feat is pre-scaled by inv on DVE
  (fp16 4x mode), the mean term goes into the fc1 activation bias
  (b1p - inv*mu*S1g), so fc1 tanh needs only a bias AP (a scale AP costs
  ~83ns/activation extra on the scalar engine).
- sigmoid rewritten as 0.5 + 0.5*tanh(x/2): the whole kernel runs on the
  tanh table until the finale's Sin; the affine folds into the finale.
- per-quad stats (bn_stats -> cross-partition reduce -> quake-rsqrt Newton)
  batched across both branches; quad 0 runs per-branch chains to shorten
  the head.
- finale: one batched block over [128, 2T] fp16 at the end; fp16 outputs
  (host upcasts). Freq row 256 assembled on host from the aux mask rows.
"""
import sys
sys.path.insert(0, '/opt/trn_rl_repo')

import numpy as np
import ml_dtypes

import concourse.bacc as bacc
import concourse.tile as tile
import concourse.mybir as mybir
from concourse.bass_utils import run_bass_kernel_spmd

F32 = mybir.dt.float32
FP16 = mybir.dt.float16
I32 = mybir.dt.int32
AF = mybir.ActivationFunctionType
ALU = mybir.AluOpType

# problem constants (hardcoded per contract)
B, C, T = 8, 128, 512
BANDS = [2] + [3] * 10 + [8] * 12 + [16] * 7 + [17]
K = len(BANDS)                      # 31
F = sum(BANDS)                      # 257
H = 4 * C                           # 512
NHC = H // 128                      # 4 h-chunks
EPS = 1e-5

OFFS = np.concatenate([[0], np.cumsum(BANDS)]).astype(int)   # band start freqs
WPADS = [w + (w & 1) for w in BANDS]                         # even-M pad
WOFFS = np.concatenate([[0], np.cumsum(WPADS)]).astype(int)
WPTOT = int(WOFFS[-1])

QUADS = [(4 * i, 4) for i in range(7)] + [(28, 3)]
NQ = len(QUADS)
MAGIC16 = 1536.0                     # 1.5 * 2**10: fp16 round-to-int magic
INV2PI = float(1.0 / (2 * np.pi))
N2PI = float(-2 * np.pi)
PI = float(np.pi)
N_WARM = 6

_cache = {}


def _prep_branch(gamma, beta, W1, b1, W2, b2, is_mag):
    """Host-side constant prep for one branch."""
    # W1gT[c, k*H + h] = W1[k,h,c] * gamma[k,c]
    W1g = W1 * gamma[:, None, :]                      # [K, H, C]
    W1gT = np.ascontiguousarray(W1g.transpose(2, 0, 1).reshape(C, K * H))
    W1gT16 = W1gT.astype(np.float16)
    # b1p[k,h] = b1[k,h] + sum_c W1[k,h,c]*beta[k,c];  layout [128, K*NHC]
    b1p = b1 + np.einsum('khc,kc->kh', W1, beta)      # [K, H]
    b1pT = np.zeros((128, K * NHC), np.float32)
    # ns1gt[p, k*NHC+hc] = -sum_c W1g16[c, k, hc*128+p]
    ns1gt = np.zeros((128, K * NHC), np.float32)
    W1gT16f = W1gT16.astype(np.float32)
    for k in range(K):
        for hc in range(NHC):
            b1pT[:, k * NHC + hc] = b1p[k, hc * 128:(hc + 1) * 128]
            ns1gt[:, k * NHC + hc] = -W1gT16f[:, k * H + hc * 128:
                                              k * H + (hc + 1) * 128].sum(axis=0)
    # mag branch computes y = tanh(0.5*(fc2 + b2)): fold 0.5 into W2 and b2
    wscale = 0.5 if is_mag else 1.0
    W2Tp = np.zeros((128, NHC * WPTOT), np.float32)
    for k in range(K):
        w, off, woff = BANDS[k], OFFS[k], WOFFS[k]
        for hc in range(NHC):
            W2Tp[:, hc * WPTOT + woff: hc * WPTOT + woff + w] = \
                wscale * W2[off:off + w, hc * 128:(hc + 1) * 128].T
    W2Tp = W2Tp.astype(np.float16)
    b2g = np.zeros((128, len(QUADS)), np.float32)
    for q, (k0, nb) in enumerate(QUADS):
        for r in range(nb):
            k = k0 + r
            b2g[32 * r:32 * r + BANDS[k], q] = \
                wscale * b2[OFFS[k]:OFFS[k] + BANDS[k]]
    return W1gT16, b1pT, ns1gt, W2Tp, b2g


def _build():
    nc = bacc.Bacc("TRN2", target_bir_lowering=False)

    # per-core inputs
    ins = {}
    for br in ("m", "p"):
        ins[f"feat_{br}"] = nc.dram_tensor(f"feat_{br}", [C, K * T], FP16,
                                           kind="ExternalInput")
        ins[f"w1gt_{br}"] = nc.dram_tensor(f"w1gt_{br}", [C, K * H], FP16,
                                           kind="ExternalInput")
        ins[f"b1pt_{br}"] = nc.dram_tensor(f"b1pt_{br}", [128, K * NHC], F32,
                                           kind="ExternalInput")
        ins[f"ns1gt_{br}"] = nc.dram_tensor(f"ns1gt_{br}", [128, K * NHC], F32,
                                            kind="ExternalInput")
        ins[f"w2tp_{br}"] = nc.dram_tensor(f"w2tp_{br}", [128, NHC * WPTOT],
                                           FP16, kind="ExternalInput")
        ins[f"b2c_{br}"] = nc.dram_tensor(f"b2c_{br}", [128, len(QUADS)], F32,
                                          kind="ExternalInput")
        ins[f"noisy_{br}"] = nc.dram_tensor(f"noisy_{br}", [2 * 128, T], FP16,
                                            kind="ExternalInput")
    halfpi_d = nc.dram_tensor("halfpi", [128, 1], F32, kind="ExternalInput")
    out_d = nc.dram_tensor("out", [2 * 128, 2 * T], FP16,
                           kind="ExternalOutput")
    aux_d = nc.dram_tensor("aux", [2, T], FP16, kind="ExternalOutput")

    with tile.TileContext(nc) as tc:
        with (
            tc.tile_pool(name="featk", bufs=4) as featk_pool,
            tc.tile_pool(name="fsc", bufs=4) as fsc_pool,
            tc.tile_pool(name="w1t", bufs=4) as w1t_pool,
            tc.tile_pool(name="h1sb", bufs=3) as h1sb_pool,
            tc.tile_pool(name="band", bufs=6) as band_pool,
            tc.tile_pool(name="const", bufs=1) as const_pool,
            tc.tile_pool(name="statsb", bufs=2) as stats_pool,
            tc.tile_pool(name="fin", bufs=1) as fin_pool,
            tc.tile_pool(name="mainps", bufs=1, space="PSUM") as main_ps,
        ):
            # ---- critical-path first: quad-0 fetches before anything ----
            k0_0, nb_0 = QUADS[0]
            eng_f = {"m": nc.sync, "p": nc.gpsimd}
            fq0 = {}
            wq0 = {}
            for br in ("m", "p"):
                fq0[br] = featk_pool.tile([128, nb_0 * T], FP16, tag="featq",
                                          name=f"featq_{br}_0")
                # per-band DMAs so band-0 stats can start early
                for r in range(nb_0):
                    eng_f[br].dma_start(
                        fq0[br][:, r * T:(r + 1) * T],
                        ins[f"feat_{br}"][:, (k0_0 + r) * T:(k0_0 + r + 1) * T])
                wq0[br] = w1t_pool.tile([128, nb_0 * H], FP16, tag="w1q",
                                        name=f"w1q_{br}_0")
                nc.scalar.dma_start(
                    wq0[br][:], ins[f"w1gt_{br}"][:, k0_0 * H:(k0_0 + nb_0) * H])

            # ---- all-ones stationary for the reduce-broadcast matmul ----
            # fp16 (with fp16 sums) keeps the per-quad ps_r matmul off the
            # slow f32 weight-load path (~600ns -> ~150ns)
            ones128 = const_pool.tile([128, 128], FP16, tag="ones128",
                                      name="ones128")
            nc.vector.memset(ones128[:], 1.0)
            # ---- PE warm-up from a memset tile (no DMA dependency) ----
            warm_sb = const_pool.tile([128, T], FP16, tag="warm_sb",
                                      name="warm_sb")
            nc.vector.memset(warm_sb[:], 0.001)
            for wi in range(N_WARM):
                wps = main_ps.tile([128, T], F32, tag="h1ps", bufs=5,
                                   name=f"warm_{wi}")
                nc.tensor.matmul(wps[:], warm_sb[:, 0:128], warm_sb[:],
                                 start=True, stop=True)

            cb = {}
            noisy = {}
            for br in ("m", "p"):
                b1pt = const_pool.tile([128, K * NHC], F32, tag=f"b1pt_{br}",
                                       name=f"b1pt_{br}")
                nc.scalar.dma_start(b1pt[:], ins[f"b1pt_{br}"][:])
                ns1gt = const_pool.tile([128, K * NHC], F32, tag=f"ns1gt_{br}",
                                        name=f"ns1gt_{br}")
                nc.scalar.dma_start(ns1gt[:], ins[f"ns1gt_{br}"][:])
                w2tp = const_pool.tile([128, NHC * WPTOT], FP16,
                                       tag=f"w2tp_{br}", name=f"w2tp_{br}")
                nc.gpsimd.dma_start(w2tp[:], ins[f"w2tp_{br}"][:])
                b2c = const_pool.tile([128, len(QUADS)], F32, tag=f"b2c_{br}",
                                      name=f"b2c_{br}")
                nc.sync.dma_start(b2c[:], ins[f"b2c_{br}"][:])
                cb[br] = (b1pt, ns1gt, w2tp, b2c)
                nz = const_pool.tile([128, 2 * T], FP16, tag=f"noisy_{br}",
                                     name=f"noisy_{br}")
                noisy[br] = nz
            halfpi = const_pool.tile([128, 1], F32)

            # ---- masks (tanh outputs y), fp16 ----
            masks = {}
            for br in ("m", "p"):
                masks[br] = const_pool.tile([128, 2 * T], FP16,
                                            tag=f"mask_{br}", name=f"mask_{br}")
                masks[br + "2"] = const_pool.tile([1, T], FP16,
                                                  tag=f"mask2_{br}",
                                                  name=f"mask2_{br}")

            def stats_chain(q, k0, nb, fq, branches):
                """bn stats -> reduce-broadcast -> newton rsqrt -> prescaled
                feat + per-(band,hc) activation bias for `branches`.

                Returns ({br: fsc}, {br: biasq}).
                """
                nbr = len(branches)
                sfx = f"{q}_{branches[0]}"
                sums = stats_pool.tile([128, 2 * nb * nbr], FP16, tag="sums",
                                       bufs=3, name=f"sums_{sfx}")
                for bi, br in enumerate(branches):
                    o = 2 * nb * bi
                    st_q = stats_pool.tile([128, nb * 6], F32, tag="st_q",
                                           name=f"st_{br}_{q}")
                    ag_q = stats_pool.tile([128, nb * 2], F32, tag="ag_q",
                                           name=f"ag_{br}_{q}")
                    for r in range(nb):
                        nc.vector.bn_stats(st_q[:, r * 6:(r + 1) * 6],
                                           fq[br][:, r * T:(r + 1) * T])
                        nc.vector.bn_aggr(ag_q[:, r * 2:(r + 1) * 2],
                                          st_q[:, r * 6:(r + 1) * 6])
                    ag3 = ag_q[:].rearrange("c (k two) -> c k two", two=2)
                    mean_ap = ag3[:, :, 0]
                    var_ap = ag3[:, :, 1]
                    nc.vector.tensor_copy(sums[:, o:o + nb], mean_ap)
                    tmp = stats_pool.tile([128, nb], F32, tag="tmp",
                                          name=f"tmp_{br}_{q}")
                    nc.vector.tensor_mul(tmp[:], mean_ap, mean_ap)
                    nc.vector.tensor_add(sums[:, o + nb:o + 2 * nb], tmp[:],
                                         var_ap)
                # reduce + broadcast in one matmul: out[m, j] = sum_p sums[p,j]
                ps_r = main_ps.tile([128, 2 * nb * nbr], F32, tag="ps_s",
                                    bufs=1, name=f"ps_r_{sfx}")
                nc.tensor.matmul(ps_r[:], ones128[:], sums[:],
                                 start=True, stop=True)
                g = stats_pool.tile([128, 2 * nb * nbr], F32, tag="g",
                                    name=f"g_{sfx}")
                nc.vector.tensor_scalar_mul(g[:], ps_r[:], 1.0 / C)
                g4 = g[:].rearrange("o (b two n) -> o b two n", b=nbr, two=2)
                gmean = g4[:, :, 0, :]
                gsq = g4[:, :, 1, :]
                gm2 = stats_pool.tile([128, nb * nbr], F32, tag="gm2",
                                      name=f"gm2_{sfx}")
                nc.vector.tensor_mul(gm2[:], gmean, gmean)
                vv = stats_pool.tile([128, nb * nbr], F32, tag="vv",
                                     name=f"vv_{sfx}")
                nc.vector.tensor_sub(vv[:], gsq, gm2[:])
                nc.vector.tensor_scalar_add(vv[:], vv[:], EPS)
                yy = stats_pool.tile([128, nb * nbr], F32, tag="yy",
                                     name=f"yy_{sfx}")
                nc.vector.tensor_scalar(yy[:].bitcast(I32), vv[:].bitcast(I32),
                                        1, -1, op0=ALU.arith_shift_right,
                                        op1=ALU.bitwise_xor)
                nc.vector.tensor_scalar_add(yy[:].bitcast(I32),
                                            yy[:].bitcast(I32), 0x5f3759e0)
                bbq = stats_pool.tile([128, 2 * nb * nbr], F32, tag="bbq",
                                      bufs=3, name=f"bbq_{sfx}")
                iv4 = bbq[:].rearrange("o (b two n) -> o b two n",
                                       b=nbr, two=2)
                inv_ap = iv4[:, :, 0, :]
                invmu_ap = iv4[:, :, 1, :]
                tnr = stats_pool.tile([128, nb * nbr], F32, tag="tnr",
                                      name=f"tnr_{sfx}")
                for it in range(3):
                    nc.vector.tensor_mul(tnr[:], yy[:], yy[:])
                    nc.vector.tensor_mul(tnr[:], tnr[:], vv[:])
                    nc.vector.tensor_scalar(tnr[:], tnr[:], -0.5, 1.5,
                                            op0=ALU.mult, op1=ALU.add)
                    dst = yy[:] if it < 2 else inv_ap
                    nc.vector.tensor_mul(dst, yy[:], tnr[:])
                nc.vector.tensor_mul(invmu_ap, inv_ap, gmean)

                fscs = {}
                biasqs = {}
                for bi, br in enumerate(branches):
                    o = 2 * nb * bi
                    b1pt, ns1gt, _, _ = cb[br]
                    fsc = fsc_pool.tile([128, nb * T], FP16, tag="fsc",
                                        name=f"fsc_{br}_{q}")
                    for r in range(nb):
                        nc.vector.tensor_scalar_mul(
                            fsc[:, r * T:(r + 1) * T],
                            fq[br][:, r * T:(r + 1) * T],
                            bbq[:, o + r:o + r + 1])
                    fscs[br] = fsc
                    biasq = stats_pool.tile([128, nb * NHC], F32, tag="biasq",
                                            bufs=3, name=f"biasq_{br}_{q}")
                    for r in range(nb):
                        k = k0 + r
                        nc.vector.scalar_tensor_tensor(
                            biasq[:, r * NHC:(r + 1) * NHC],
                            ns1gt[:, k * NHC:(k + 1) * NHC],
                            bbq[:, o + nb + r:o + nb + r + 1],
                            b1pt[:, k * NHC:(k + 1) * NHC],
                            op0=ALU.mult, op1=ALU.add)
                    biasqs[br] = biasq
                return fscs, biasqs

            # ---- fused per-quad stats + band pipeline ----
            for q, (k0, nb) in enumerate(QUADS):
                fq = {}
                wq = {}
                for br in ("m", "p"):
                    if q == 0:
                        fq[br], wq[br] = fq0[br], wq0[br]
                    else:
                        fq[br] = featk_pool.tile([128, nb * T], FP16,
                                                 tag="featq",
                                                 name=f"featq_{br}_{q}")
                        nc.sync.dma_start(
                            fq[br][:], ins[f"feat_{br}"][:, k0 * T:(k0 + nb) * T])
                        wq[br] = w1t_pool.tile([128, nb * H], FP16, tag="w1q",
                                               name=f"w1q_{br}_{q}")
                        nc.sync.dma_start(
                            wq[br][:], ins[f"w1gt_{br}"][:, k0 * H:(k0 + nb) * H])

                if q == 0:
                    fscs, biasqs = {}, {}
                    for br in ("m", "p"):
                        fs, bq = stats_chain(q, k0, nb, fq, (br,))
                        fscs.update(fs)
                        biasqs.update(bq)
                    # second warm-up burst: bridges the PE gap between the
                    # head warm-up and the first real fc1 so the clock
                    # doesn't fall back to the mid p-state
                    for wi in range(6):
                        wps = main_ps.tile([128, T], F32, tag="h1ps", bufs=5,
                                           name=f"warm2_{wi}")
                        nc.tensor.matmul(wps[:], warm_sb[:, 0:128],
                                         warm_sb[:], start=True, stop=True)
                else:
                    fscs, biasqs = stats_chain(q, k0, nb, fq, ("m", "p"))

                if q == 2:
                    nc.gpsimd.dma_start(halfpi[:], halfpi_d[:])
                    for br in ("m", "p"):
                        for j in range(2):
                            nc.gpsimd.dma_start(
                                noisy[br][:, j * T:(j + 1) * T],
                                ins[f"noisy_{br}"][j * 128:(j + 1) * 128, :])

                for br in ("m", "p"):
                    b1pt, ns1gt, w2tp, b2c = cb[br]
                    biasq = biasqs[br]
                    fsc = fscs[br]
                    h1s = []
                    for r in range(nb):
                        k = k0 + r
                        h1sb = h1sb_pool.tile([128, NHC * T], FP16, bufs=8)
                        h1s.append(h1sb)
                        for hc in range(NHC):
                            h1ps = main_ps.tile([128, T], F32, tag="h1ps",
                                                bufs=5,
                                                name=f"h1ps_{br}_{k}_{hc}")
                            nc.tensor.matmul(
                                h1ps[:],
                                wq[br][:, (r * NHC + hc) * 128:
                                       (r * NHC + hc + 1) * 128],
                                fsc[:, r * T:(r + 1) * T],
                                start=True, stop=True)
                            nc.scalar.activation(
                                h1sb[:, hc * T:(hc + 1) * T], h1ps[:],
                                AF.Tanh,
                                bias=biasq[:, r * NHC + hc:r * NHC + hc + 1])
                    fc2g = main_ps.tile([128, T], F32, tag="fc2ps", bufs=2,
                                        name=f"fc2g_{br}_{q}")
                    for r in range(nb):
                        k = k0 + r
                        wp, woff = WPADS[k], int(WOFFS[k])
                        for hc in range(NHC):
                            nc.tensor.matmul(
                                fc2g[32 * r:32 * r + wp, :],
                                w2tp[:, hc * WPTOT + woff:
                                     hc * WPTOT + woff + wp],
                                h1s[r][:, hc * T:(hc + 1) * T],
                                start=(hc == 0), stop=(hc == NHC - 1),
                                tile_position=(0, 32 * r))
                    grp_t = band_pool.tile([128, T], FP16, tag="band")
                    nc.scalar.activation(grp_t[:], fc2g[:], AF.Tanh,
                                         bias=b2c[:, q:q + 1])
                    for r in range(nb):
                        k = k0 + r
                        w, off = BANDS[k], int(OFFS[k])
                        j0, r0 = off // 128, off % 128
                        if off + w <= (j0 + 1) * 128:
                            nc.gpsimd.dma_start(
                                masks[br][r0:r0 + w, j0 * T:(j0 + 1) * T],
                                grp_t[32 * r:32 * r + w, :])
                        else:
                            n1 = (j0 + 1) * 128 - off
                            nc.gpsimd.dma_start(
                                masks[br][r0:128, j0 * T:(j0 + 1) * T],
                                grp_t[32 * r:32 * r + n1, :])
                            rem = w - n1
                            if j0 + 1 < 2:
                                nc.gpsimd.dma_start(
                                    masks[br][0:rem, (j0 + 1) * T:(j0 + 2) * T],
                                    grp_t[32 * r + n1:32 * r + w, :])
                            else:
                                nc.gpsimd.dma_start(
                                    masks[br + "2"][0:rem, :],
                                    grp_t[32 * r + n1:32 * r + w, :])

            # ---- batched finale over both f-chunks [128, 2T] ----
            W2T = 2 * T
            y_m = masks["m"][:]
            y_p = masks["p"][:]
            ang = fin_pool.tile([128, W2T], FP16, tag="ang", name="ang")
            nc.vector.scalar_tensor_tensor(ang[:], y_p, PI, noisy["p"][:],
                                           op0=ALU.mult, op1=ALU.add)
            t2 = fin_pool.tile([128, W2T], FP16, tag="t2", name="t2")
            nc.vector.tensor_scalar(t2[:], ang[:], INV2PI, MAGIC16,
                                    op0=ALU.mult, op1=ALU.add)
            m2pin = fin_pool.tile([128, W2T], FP16, tag="m2pin", name="m2pin")
            nc.vector.tensor_scalar(m2pin[:], t2[:], MAGIC16, N2PI,
                                    op0=ALU.subtract, op1=ALU.mult)
            nc.vector.tensor_add(m2pin[:], ang[:], m2pin[:])
            sn = fin_pool.tile([128, W2T], FP16, tag="sn", name="sn")
            nc.scalar.activation(sn[:], m2pin[:], AF.Sin)
            t2c = fin_pool.tile([128, W2T], FP16, tag="t2c", name="t2c")
            nc.vector.tensor_scalar(t2c[:], ang[:], INV2PI, 0.25,
                                    op0=ALU.mult, op1=ALU.add)
            nc.vector.tensor_scalar_add(t2c[:], t2c[:], MAGIC16)
            m2pinc = fin_pool.tile([128, W2T], FP16, tag="m2pinc",
                                   name="m2pinc")
            nc.vector.tensor_scalar(m2pinc[:], t2c[:], MAGIC16, N2PI,
                                    op0=ALU.subtract, op1=ALU.mult)
            nc.vector.tensor_add(m2pinc[:], ang[:], m2pinc[:])
            cn = fin_pool.tile([128, W2T], FP16, tag="cn", name="cn")
            nc.scalar.activation(cn[:], m2pinc[:], AF.Sin, bias=halfpi[:])
            mk = fin_pool.tile([128, W2T], FP16, tag="mk", name="mk")
            nc.vector.tensor_scalar(mk[:], y_m, 0.5, 0.5,
                                    op0=ALU.mult, op1=ALU.add)
            enh = fin_pool.tile([128, W2T], FP16, tag="enh", name="enh")
            nc.vector.tensor_mul(enh[:], mk[:], noisy["m"][:])
            ot = fin_pool.tile([128, 2 * W2T], FP16, tag="ot", name="ot")
            ot4 = ot[:].rearrange("p (j two t) -> p j two t", j=2, two=2)
            enh2 = enh[:].rearrange("p (j t) -> p j t", j=2)
            cn2 = cn[:].rearrange("p (j t) -> p j t", j=2)
            sn2 = sn[:].rearrange("p (j t) -> p j t", j=2)
            nc.vector.tensor_mul(ot4[:, :, 0, :], enh2, cn2)
            nc.vector.tensor_mul(ot4[:, :, 1, :], enh2, sn2)
            for j in range(2):
                nc.sync.dma_start(out_d[j * 128:(j + 1) * 128, :],
                                  ot[:, j * W2T:(j + 1) * W2T])
            nc.sync.dma_start(aux_d[0:1, :], masks["m2"][:])
            nc.sync.dma_start(aux_d[1:2, :], masks["p2"][:])

    nc.compile()
    return nc


def kernel(mag_features, phase_features, noisy_mag, noisy_phase,
           mag_gamma, mag_beta, mag_W1, mag_b1, mag_W2, mag_b2,
           ph_gamma, ph_beta, ph_W1, ph_b1, ph_W2, ph_b2):
    if "nc" not in _cache:
        _cache["nc"] = _build()
    nc = _cache["nc"]

    mW1gT, mb1pT, mns1gt, mW2Tp, mb2c = _prep_branch(
        np.asarray(mag_gamma), np.asarray(mag_beta), np.asarray(mag_W1),
        np.asarray(mag_b1), np.asarray(mag_W2), np.asarray(mag_b2),
        is_mag=True)
    pW1gT, pb1pT, pns1gt, pW2Tp, pb2c = _prep_branch(
        np.asarray(ph_gamma), np.asarray(ph_beta), np.asarray(ph_W1),
        np.asarray(ph_b1), np.asarray(ph_W2), np.asarray(ph_b2),
        is_mag=False)

    shared = dict(
        w1gt_m=mW1gT, b1pt_m=mb1pT, ns1gt_m=mns1gt, w2tp_m=mW2Tp, b2c_m=mb2c,
        w1gt_p=pW1gT, b1pt_p=pb1pT, ns1gt_p=pns1gt, w2tp_p=pW2Tp, b2c_p=pb2c,
        halfpi=np.full((128, 1), np.pi / 2, np.float32),
    )
    mag_features = np.asarray(mag_features)
    phase_features = np.asarray(phase_features)
    noisy_mag = np.asarray(noisy_mag, dtype=np.float32)
    noisy_phase = np.asarray(noisy_phase, dtype=np.float32)

    in_maps = []
    for b in range(B):
        m = dict(shared)
        # [C, T, K] -> [C, K, T] k-major, contiguous per-band slices; fp16
        m["feat_m"] = np.ascontiguousarray(
            mag_features[b].transpose(0, 2, 1)).reshape(C, K * T).astype(
                np.float16)
        m["feat_p"] = np.ascontiguousarray(
            phase_features[b].transpose(0, 2, 1)).reshape(C, K * T).astype(
                np.float16)
        m["noisy_m"] = np.ascontiguousarray(noisy_mag[b][0:256]).astype(
            np.float16)
        m["noisy_p"] = np.ascontiguousarray(noisy_phase[b][0:256]).astype(
            np.float16)
        in_maps.append(m)

    import os
    trace = bool(os.environ.get("BASS_PROFILE"))
    res = run_bass_kernel_spmd(nc, in_maps, list(range(B)), trace=trace)
    _cache["last_result"] = res

    out = np.empty((B, F, T), np.complex64)
    for b in range(B):
        o = res.results[b]["out"].astype(np.float32)   # [256, 2T] fp16
        out[b, 0:256].real = o[:, 0:T]
        out[b, 0:256].imag = o[:, T:2 * T]
        aux = res.results[b]["aux"].astype(np.float32)  # [2, T] fp16
        y_m, y_p = aux[0], aux[1]
        mask = 0.5 + 0.5 * y_m
        enh = noisy_mag[b, 256] * mask
        ang = noisy_phase[b, 256] + np.float32(np.pi) * y_p
        out[b, 256] = (enh * np.exp(1j * ang)).astype(np.complex64)
    return out
